# revision 4
# baseline (speedup 1.0000x reference)
"""Online Normalization forward (nn_Norm1d) on 8 Trainium2 NeuronCores.

Reference recurrence over the batch dim t (per feature, sequential):
    d_t   = x_t - mu_t
    y_t   = d_t / sqrt(var_t + eps)
    mu_{t+1}  = mu_t + (1-a)*d_t
    var_{t+1} = a*var_t + a*(1-a)*d_t^2

Sharding: tensor-parallel over the feature dim L (4096 -> 8 x 512); each
feature's scan over N=8192 is independent, so no cross-core communication.

Kernel structure (per core, 512 features):
  - fp16 I/O: host casts x to fp16 (halves HBM reads), y stored fp16 and
    upcast on host (halves HBM writes).
  - 127-step blocks; time lives on SBUF partitions 0..126, the running
    carry (mu or var) rides partition 127 of the same moving tile, so ONE
    matmul per block per path computes all 127 outputs AND the next carry.
  - The serial block-to-block carry chain is broken with a block-level
    scan: per-block carry increments m(b) are computed by independent
    "m-matmuls" that accumulate into one PSUM tile (stationary with only
    column b-mod-8 nonzero), then one small scan-matmul (lower-triangular
    in A = a^127) produces all 8 carries of a super-block at once, and a
    single SBUF->SBUF DMA scatters them into partition 127 of the x/e
    tiles.  Same machinery twice: mu-path (from x) and var-path (from
    e = d^2, which only exists after the d-pass -> the v-pass trails one
    super-block behind; the Tile scheduler overlaps them).
  - Elementwise work: dcopy (PSUM->SBUF fp16, overwrites the consumed x
    block in place), square, y-mul on the vector engine (2x fp16 modes);
    rsqrt (Abs_reciprocal_sqrt table) on the scalar engine.
  - DMA dispatch spread: x-loads on gpsimd (SWDGE), y-stores and scatters
    on sync (HWDGE), so neither compute engine pays dispatch time.
"""

import sys

for _p in ("/opt/trn_rl_repo", "/root/.axon_site/_ro/trn_rl_repo"):
    if _p not in sys.path:
        sys.path.append(_p)

import numpy as np

import concourse.bacc as bacc
import concourse.mybir as mybir
from concourse.tile import TileContext
from concourse import bass_utils

N_ROWS = 8192
L_FULL = 4096
N_CORES = 8
L_SHARD = L_FULL // N_CORES

AFWD = 0.999
EPS = 1e-05
B = 127          # time steps per block (partition 127 = carry row)
NBLK = 65        # 64 full blocks + 64-row tail
SB = 8           # blocks per super-block
NSB = 8          # super-blocks covering blocks 0..63
TAIL_ROWS = N_ROWS - 64 * B  # 64

F32 = mybir.dt.float32
F16 = mybir.dt.float16
AF = mybir.ActivationFunctionType


def _build_weights():
    A = AFWD
    # WD [128,128]: [j,k] = coeff of moving row j for output col k.
    WD = np.zeros((128, 128))
    for k in range(B):
        WD[k, k] = 1.0
        for j in range(k):
            WD[j, k] = -(1 - A) * A ** (k - 1 - j)
        WD[127, k] = -(A ** k)
    for j in range(B):
        WD[j, 127] = (1 - A) * A ** (126 - j)
    WD[127, 127] = A ** 127

    # TV [128,128]: v^(k) = a^k*nu + (1-a) sum_{j<k} a^(k-j) e_j
    TV = np.zeros((128, 128))
    for k in range(B):
        for j in range(k):
            TV[j, k] = (1 - A) * A ** (k - j)
        TV[127, k] = A ** k
    for j in range(B):
        TV[j, 127] = (1 - A) * A ** (127 - j)
    TV[127, 127] = A ** 127

    # WM8 [127, 64]: 8 stacked [127,8] stationaries; slot i has only col i
    # nonzero = per-block mu increment coefficients.
    WM8 = np.zeros((B, SB * SB))
    WMv8 = np.zeros((B, SB * SB))
    for i in range(SB):
        for j in range(B):
            WM8[j, SB * i + i] = (1 - A) * A ** (126 - j)
            WMv8[j, SB * i + i] = (1 - A) * A ** (127 - j)

    # LS [9,9]: block-level scan in Abig = a^127.
    Abig = A ** B
    LS = np.zeros((SB + 1, SB + 1))
    for k in range(SB + 1):
        for i in range(min(k, SB)):
            LS[i, k] = Abig ** (k - 1 - i)
        LS[SB, k] = Abig ** k
    return {"wd": WD, "tv": TV, "wm8": WM8, "wmv8": WMv8, "ls": LS}


_WEIGHTS = {k: np.ascontiguousarray(v.astype(np.float16))
            for k, v in _build_weights().items()}


def _build_nc(l_cols: int):
    nc = bacc.Bacc()
    x = nc.declare_dram_parameter("x", [N_ROWS, l_cols], F16, isOutput=False)
    mu0 = nc.declare_dram_parameter("mu0", [1, l_cols], F16, isOutput=False)
    var0 = nc.declare_dram_parameter("var0", [1, l_cols], F16, isOutput=False)
    wts = {
        name: nc.declare_dram_parameter(name, list(w.shape), F16,
                                        isOutput=False)
        for name, w in _WEIGHTS.items()
    }
    y = nc.declare_dram_parameter("y", [N_ROWS, l_cols], F16, isOutput=True)

    CW = SB * 512  # columns per super-block tile (4096)

    with TileContext(nc) as tc:
        with (
            tc.tile_pool(name="consts", bufs=1) as cpool,
            tc.tile_pool(name="xsb", bufs=3) as xsb_pool,
            tc.tile_pool(name="esb", bufs=2) as esb_pool,
            tc.tile_pool(name="ysb", bufs=2) as ysb_pool,
            tc.tile_pool(name="rs", bufs=3) as rs_pool,
            tc.tile_pool(name="smov", bufs=2) as smu_pool,
            tc.tile_pool(name="smovv", bufs=2) as smv_pool,
            tc.tile_pool(name="ct", bufs=2) as ct_pool,
            tc.tile_pool(name="cvt", bufs=2) as cvt_pool,
            tc.tile_pool(name="pd", bufs=2, space="PSUM") as pd_pool,
            tc.tile_pool(name="pv", bufs=2, space="PSUM") as pv_pool,
            tc.tile_pool(name="pm", bufs=1, space="PSUM") as pm_pool,
            tc.tile_pool(name="pmv", bufs=1, space="PSUM") as pmv_pool,
            tc.tile_pool(name="pc", bufs=1, space="PSUM") as pc_pool,
            tc.tile_pool(name="pcv", bufs=1, space="PSUM") as pcv_pool,
        ):
            wsb = {}
            for name, w in _WEIGHTS.items():
                wsb[name] = cpool.tile(list(w.shape), F16, tag=name,
                                       name=f"w_{name}")
                nc.sync.dma_start(out=wsb[name][:, :], in_=wts[name][:, :])
            eps_sb = cpool.tile([128, 1], F32, tag="eps", name="eps_sb")
            nc.vector.memset(eps_sb[:, :], EPS)

            # tail block tiles (block 64: rows 8128..8191, 64 valid rows)
            xtail = cpool.tile([128, 512], F16, tag="xtail", name="xtail")
            etail = cpool.tile([128, 512], F16, tag="etail", name="etail")
            nc.vector.memset(xtail[64:128, :], 0.0)

            smu = [None] * (NSB + 1)
            smv = [None] * (NSB + 1)
            smu[0] = smu_pool.tile([SB + 1, 512], F16, tag="smu", name="smu0")
            smv[0] = smv_pool.tile([SB + 1, 512], F16, tag="smv", name="smv0")
            nc.sync.dma_start(out=smu[0][SB:SB + 1, :], in_=mu0[:, :])
            nc.sync.dma_start(out=smv[0][SB:SB + 1, :], in_=var0[:, :])

            xsbs = [None] * NSB
            esbs = [None] * NSB

            for s in range(NSB):
                # ---------- mu prologue ----------
                xsb = xsb_pool.tile([128, CW], F16, tag="xsb",
                                    name=f"xsb{s}")
                xsbs[s] = xsb
                for q in range(4):  # pairs within the super-block
                    p = 4 * s + q
                    src = x[254 * p:254 * p + 254, :].rearrange(
                        "(c p) f -> p c f", c=2)
                    nc.gpsimd.dma_start(
                        out=xsb[0:B, 1024 * q:1024 * q + 1024], in_=src)
                pm = pm_pool.tile([SB, 512], F32, tag="pm", name=f"pm{s}")
                for i in range(SB):
                    nc.tensor.matmul(pm[0:SB, :],
                                     wsb["wm8"][:, SB * i:SB * i + SB],
                                     xsb[0:B, 512 * i:512 * i + 512],
                                     start=(i == 0), stop=(i == SB - 1))
                nc.vector.tensor_copy(smu[s][0:SB, :], pm[0:SB, :])
                pc = pc_pool.tile([SB + 1, 512], F32, tag="pc", name=f"pc{s}")
                nc.tensor.matmul(pc[0:SB + 1, :], wsb["ls"][:, :],
                                 smu[s][0:SB + 1, :], start=True, stop=True)
                ct = ct_pool.tile([SB + 1, 512], F16, tag="ct", name=f"ct{s}")
                nc.vector.tensor_copy(ct[0:SB + 1, :], pc[0:SB + 1, :])
                # scatter mu carries into partition 127 of xsb
                nc.sync.dma_start(out=xsb[127:128, 0:CW], in_=ct[0:SB, :])
                if s < NSB - 1:
                    smu[s + 1] = smu_pool.tile([SB + 1, 512], F16, tag="smu",
                                               name=f"smu{s + 1}")
                    nc.sync.dma_start(out=smu[s + 1][SB:SB + 1, :],
                                      in_=ct[SB:SB + 1, :])
                else:
                    nc.sync.dma_start(out=xtail[127:128, :],
                                      in_=ct[SB:SB + 1, :])

                # ---------- d pass ----------
                esb = esb_pool.tile([128, CW], F16, tag="esb",
                                    name=f"esb{s}")
                esbs[s] = esb
                pmv = pmv_pool.tile([SB, 512], F32, tag="pmv", name=f"pmv{s}")
                for i in range(SB):
                    cols = slice(512 * i, 512 * i + 512)
                    pd = pd_pool.tile([128, 512], F32, tag="pd")
                    nc.tensor.matmul(pd[:, :], wsb["wd"][:, :],
                                     xsb[0:128, cols], start=True, stop=True)
                    # d overwrites the consumed x block in place (fp16)
                    nc.vector.tensor_copy(xsb[0:B, cols], pd[0:B, :])
                    nc.vector.tensor_mul(esb[0:B, cols], xsb[0:B, cols],
                                         xsb[0:B, cols])
                    nc.tensor.matmul(pmv[0:SB, :],
                                     wsb["wmv8"][:, SB * i:SB * i + SB],
                                     esb[0:B, cols],
                                     start=(i == 0), stop=(i == SB - 1))

                # ---------- var scan ----------
                nc.vector.tensor_copy(smv[s][0:SB, :], pmv[0:SB, :])
                pcv = pcv_pool.tile([SB + 1, 512], F32, tag="pcv",
                                    name=f"pcv{s}")
                nc.tensor.matmul(pcv[0:SB + 1, :], wsb["ls"][:, :],
                                 smv[s][0:SB + 1, :], start=True, stop=True)
                cvt = cvt_pool.tile([SB + 1, 512], F16, tag="cvt",
                                    name=f"cvt{s}")
                nc.vector.tensor_copy(cvt[0:SB + 1, :], pcv[0:SB + 1, :])
                nc.sync.dma_start(out=esb[127:128, 0:CW], in_=cvt[0:SB, :])
                if s < NSB - 1:
                    smv[s + 1] = smv_pool.tile([SB + 1, 512], F16, tag="smv",
                                               name=f"smv{s + 1}")
                    nc.sync.dma_start(out=smv[s + 1][SB:SB + 1, :],
                                      in_=cvt[SB:SB + 1, :])
                else:
                    nc.sync.dma_start(out=etail[127:128, :],
                                      in_=cvt[SB:SB + 1, :])

                # ---------- v pass ----------
                ysb = ysb_pool.tile([128, CW], F16, tag="ysb",
                                    name=f"ysb{s}")
                for i in range(SB):
                    cols = slice(512 * i, 512 * i + 512)
                    pv = pv_pool.tile([128, 512], F32, tag="pv")
                    nc.tensor.matmul(pv[:, :], wsb["tv"][:, :],
                                     esb[0:128, cols], start=True, stop=True)
                    rs = rs_pool.tile([128, 512], F16, tag="rs")
                    nc.scalar.activation(rs[0:B, :], pv[0:B, :],
                                         AF.Abs_reciprocal_sqrt,
                                         bias=eps_sb[0:B, :])
                    nc.vector.tensor_mul(ysb[0:B, cols], xsb[0:B, cols],
                                         rs[0:B, :])
                dst = y[1016 * s:1016 * s + 1016, :].rearrange(
                    "(c p) f -> p c f", c=SB)
                nc.sync.dma_start(out=dst, in_=ysb[0:B, 0:CW])

            # ---------- tail block (64 rows) ----------
            nc.gpsimd.dma_start(out=xtail[0:TAIL_ROWS, :],
                                in_=x[64 * B:N_ROWS, :])
            pdt = pd_pool.tile([128, 512], F32, tag="pd", name="pdt")
            nc.tensor.matmul(pdt[:, :], wsb["wd"][:, :], xtail[0:128, :],
                             start=True, stop=True)
            nc.vector.tensor_copy(xtail[0:B, :], pdt[0:B, :])
            nc.vector.tensor_mul(etail[0:B, :], xtail[0:B, :], xtail[0:B, :])
            pvt = pv_pool.tile([128, 512], F32, tag="pv", name="pvt")
            nc.tensor.matmul(pvt[:, :], wsb["tv"][:, :], etail[0:128, :],
                             start=True, stop=True)
            rst = rs_pool.tile([128, 512], F16, tag="rs", name="rst")
            nc.scalar.activation(rst[0:B, :], pvt[0:B, :],
                                 AF.Abs_reciprocal_sqrt, bias=eps_sb[0:B, :])
            ytail = cpool.tile([128, 512], F16, tag="ytail", name="ytail")
            nc.vector.tensor_mul(ytail[0:B, :], xtail[0:B, :], rst[0:B, :])
            nc.sync.dma_start(out=y[64 * B:N_ROWS, :],
                              in_=ytail[0:TAIL_ROWS, :])

    nc.compile()
    return nc


_NC_CACHE = {}


def _get_nc():
    key = L_SHARD
    if key not in _NC_CACHE:
        _NC_CACHE[key] = _build_nc(key)
    return _NC_CACHE[key]


def kernel(x, mu0, var0, _want_time=False, _trace=False):
    x = np.asarray(x)
    mu0 = np.asarray(mu0).reshape(1, -1)
    var0 = np.asarray(var0).reshape(1, -1)
    assert x.shape == (N_ROWS, L_FULL), x.shape

    nc = _get_nc()
    in_maps = []
    for c in range(N_CORES):
        sl = slice(c * L_SHARD, (c + 1) * L_SHARD)
        in_maps.append({
            "x": np.ascontiguousarray(x[:, sl]).astype(np.float16),
            "mu0": np.ascontiguousarray(mu0[:, sl]).astype(np.float16),
            "var0": np.ascontiguousarray(var0[:, sl]).astype(np.float16),
            **_WEIGHTS,
        })

    exec_ns = None
    if _trace:
        orig_upload = bass_utils.upload_artifacts
        bass_utils.upload_artifacts = lambda tmpdir: "(skipped)"
        try:
            res = bass_utils.run_bass_kernel_spmd(
                nc, in_maps, list(range(N_CORES)), trace=True
            )
            exec_ns = res.exec_time_ns
        finally:
            bass_utils.upload_artifacts = orig_upload
    else:
        res = bass_utils.run_bass_kernel_spmd(nc, in_maps, list(range(N_CORES)))

    out = np.concatenate(
        [res.results[c]["y"] for c in range(N_CORES)], axis=1
    ).astype(np.float32)
    if _want_time:
        return out, exec_ns
    return out


# revision 13
# speedup vs baseline: 1.1475x; 1.1475x over previous
"""Online Normalization forward (nn_Norm1d) on 8 Trainium2 NeuronCores.

Reference recurrence over the batch dim t (per feature, sequential):
    d_t   = x_t - mu_t
    y_t   = d_t / sqrt(var_t + eps)
    mu_{t+1}  = mu_t + (1-a)*d_t
    var_{t+1} = a*var_t + a*(1-a)*d_t^2

Sharding: tensor-parallel over the feature dim L (4096 -> 8 x 512); each
feature's scan over N=8192 is independent, so no cross-core communication.

Kernel structure (per core, 512 features):
  - fp16 I/O: host casts x to fp16 (halves HBM reads), y stored fp16 and
    upcast on host (halves HBM writes).
  - 127-step blocks; time lives on SBUF partitions 0..126, the running
    carry (mu or var) rides partition 127 of the same moving tile, so ONE
    matmul per block per path computes all 127 outputs AND the next carry.
  - The serial block-to-block carry chain is broken with a block-level
    scan: per-block carry increments m(b) are computed by independent
    "m-matmuls" that accumulate into one PSUM tile (stationary with only
    column b-mod-8 nonzero), then one small scan-matmul (lower-triangular
    in A = a^127) produces all 8 carries of a super-block at once, and a
    single SBUF->SBUF DMA scatters them into partition 127 of the x/e
    tiles.  Same machinery twice: mu-path (from x) and var-path (from
    e = d^2, which only exists after the d-pass -> the v-pass trails one
    super-block behind; the Tile scheduler overlaps them).
  - Elementwise work: dcopy (PSUM->SBUF fp16, overwrites the consumed x
    block in place), square, y-mul on the vector engine (2x fp16 modes);
    rsqrt (Abs_reciprocal_sqrt table) on the scalar engine.
  - DMA dispatch spread: x-loads on gpsimd (SWDGE), y-stores and scatters
    on sync (HWDGE), so neither compute engine pays dispatch time.
"""

import sys

for _p in ("/opt/trn_rl_repo", "/root/.axon_site/_ro/trn_rl_repo"):
    if _p not in sys.path:
        sys.path.append(_p)

import numpy as np

import concourse.bacc as bacc
import concourse.mybir as mybir
from concourse.tile import TileContext
from concourse import bass_utils

N_ROWS = 8192
L_FULL = 4096
N_CORES = 8
L_SHARD = L_FULL // N_CORES

AFWD = 0.999
EPS = 1e-05
B = 127          # time steps per block (partition 127 = carry row)
NBLK = 65        # 64 full blocks + 64-row tail
SB = 8           # blocks per super-block
NSB = 8          # super-blocks covering blocks 0..63
TAIL_ROWS = N_ROWS - 64 * B  # 64

F32 = mybir.dt.float32
F16 = mybir.dt.float16
AF = mybir.ActivationFunctionType


def _build_weights():
    A = AFWD
    # WD [128,128]: [j,k] = coeff of moving row j for output col k.
    WD = np.zeros((128, 128))
    for k in range(B):
        WD[k, k] = 1.0
        for j in range(k):
            WD[j, k] = -(1 - A) * A ** (k - 1 - j)
        WD[127, k] = -(A ** k)
    for j in range(B):
        WD[j, 127] = (1 - A) * A ** (126 - j)
    WD[127, 127] = A ** 127

    # TV [128,128]: v^(k) = a^k*nu + (1-a) sum_{j<k} a^(k-j) e_j
    TV = np.zeros((128, 128))
    for k in range(B):
        for j in range(k):
            TV[j, k] = (1 - A) * A ** (k - j)
        TV[127, k] = A ** k
    for j in range(B):
        TV[j, 127] = (1 - A) * A ** (127 - j)
    TV[127, 127] = A ** 127

    # WM8 [127, 64]: 8 stacked [127,8] stationaries; slot i has only col i
    # nonzero = per-block mu increment coefficients.
    WM8 = np.zeros((B, SB * SB))
    WMv8 = np.zeros((B, SB * SB))
    for i in range(SB):
        for j in range(B):
            WM8[j, SB * i + i] = (1 - A) * A ** (126 - j)
            WMv8[j, SB * i + i] = (1 - A) * A ** (127 - j)

    # LS [9,9]: block-level scan in Abig = a^127.
    Abig = A ** B
    LS = np.zeros((SB + 1, SB + 1))
    for k in range(SB + 1):
        for i in range(min(k, SB)):
            LS[i, k] = Abig ** (k - 1 - i)
        LS[SB, k] = Abig ** k
    return {"wd": WD, "tv": TV, "wm8": WM8, "wmv8": WMv8, "ls": LS}


_WEIGHTS = {k: np.ascontiguousarray(v.astype(np.float16))
            for k, v in _build_weights().items()}


def _build_nc(l_cols: int):
    # x/y DRAM layout: 9 slabs of 128 rows x 4096 cols (fp16).  Slab s<8:
    # row k, col 512*i+f = x[127*(8s+i)+k, f] -- so one contiguous
    # [127, 4096] DMA loads a whole super-block.  Slab 8 (tail): cols 0:512
    # hold x rows 8128..8191 in rows 0..63, zeros elsewhere (host-built).
    nc = bacc.Bacc()
    x = nc.declare_dram_parameter("x", [9 * 128, 8 * 512], F16,
                                  isOutput=False)
    mu0 = nc.declare_dram_parameter("mu0", [1, l_cols], F16, isOutput=False)
    var0 = nc.declare_dram_parameter("var0", [1, l_cols], F16, isOutput=False)
    wts = {
        name: nc.declare_dram_parameter(name, list(w.shape), F16,
                                        isOutput=False)
        for name, w in _WEIGHTS.items()
    }
    y = nc.declare_dram_parameter("y", [9 * 128, 8 * 512], F16,
                                  isOutput=True)

    CW = SB * 512  # columns per super-block tile (4096)

    with TileContext(nc) as tc:
        with (
            tc.tile_pool(name="consts", bufs=1) as cpool,
            tc.tile_pool(name="xsb", bufs=3) as xsb_pool,
            tc.tile_pool(name="esb", bufs=2) as esb_pool,
            tc.tile_pool(name="ysb", bufs=2) as ysb_pool,
            tc.tile_pool(name="rs", bufs=3) as rs_pool,
            tc.tile_pool(name="smov", bufs=2) as smu_pool,
            tc.tile_pool(name="smovv", bufs=2) as smv_pool,
            tc.tile_pool(name="ct", bufs=2) as ct_pool,
            tc.tile_pool(name="cvt", bufs=2) as cvt_pool,
            tc.tile_pool(name="pd", bufs=2, space="PSUM") as pd_pool,
            tc.tile_pool(name="pv", bufs=2, space="PSUM") as pv_pool,
            tc.tile_pool(name="pm", bufs=1, space="PSUM") as pm_pool,
            tc.tile_pool(name="pmv", bufs=1, space="PSUM") as pmv_pool,
            tc.tile_pool(name="pc", bufs=1, space="PSUM") as pc_pool,
            tc.tile_pool(name="pcv", bufs=1, space="PSUM") as pcv_pool,
        ):
            wsb = {}
            for name, w in _WEIGHTS.items():
                wsb[name] = cpool.tile(list(w.shape), F16, tag=name,
                                       name=f"w_{name}")
                nc.sync.dma_start(out=wsb[name][:, :], in_=wts[name][:, :])
            eps_sb = cpool.tile([128, 1], F32, tag="eps", name="eps_sb")
            nc.vector.memset(eps_sb[:, :], EPS)

            # tail block tiles (block 64: rows 8128..8191, 64 valid rows;
            # host zero-pads rows 64..126 in the DRAM slab)
            xtail = cpool.tile([128, 512], F16, tag="xtail", name="xtail")
            etail = cpool.tile([128, 512], F16, tag="etail", name="etail")
            nc.gpsimd.dma_start(out=xtail[0:B, :],
                                in_=x[8 * 128:8 * 128 + B, 0:512])

            smu = [None] * (NSB + 1)
            smv = [None] * (NSB + 1)
            smu[0] = smu_pool.tile([SB + 1, 512], F16, tag="smu", name="smu0")
            smv[0] = smv_pool.tile([SB + 1, 512], F16, tag="smv", name="smv0")
            nc.sync.dma_start(out=smu[0][SB:SB + 1, :], in_=mu0[:, :])
            nc.sync.dma_start(out=smv[0][SB:SB + 1, :], in_=var0[:, :])

            xsbs = [None] * NSB
            esbs = [None] * NSB

            for s in range(NSB):
                # ---------- mu prologue ----------
                xsb = xsb_pool.tile([128, CW], F16, tag="xsb",
                                    name=f"xsb{s}")
                xsbs[s] = xsb
                nc.gpsimd.dma_start(out=xsb[0:B, :],
                                    in_=x[128 * s:128 * s + B, :])
                pm = pm_pool.tile([SB, 512], F32, tag="pm", name=f"pm{s}")
                for i in range(SB):
                    nc.tensor.matmul(pm[0:SB, :],
                                     wsb["wm8"][:, SB * i:SB * i + SB],
                                     xsb[0:B, 512 * i:512 * i + 512],
                                     start=(i == 0), stop=(i == SB - 1))
                nc.vector.tensor_copy(smu[s][0:SB, :], pm[0:SB, :])
                pc = pc_pool.tile([SB + 1, 512], F32, tag="pc", name=f"pc{s}")
                nc.tensor.matmul(pc[0:SB + 1, :], wsb["ls"][:, :],
                                 smu[s][0:SB + 1, :], start=True, stop=True)
                ct = ct_pool.tile([SB + 1, 512], F16, tag="ct", name=f"ct{s}")
                nc.vector.tensor_copy(ct[0:SB + 1, :], pc[0:SB + 1, :])
                # scatter mu carries into partition 127 of xsb
                nc.sync.dma_start(out=xsb[127:128, 0:CW], in_=ct[0:SB, :])
                if s < NSB - 1:
                    smu[s + 1] = smu_pool.tile([SB + 1, 512], F16, tag="smu",
                                               name=f"smu{s + 1}")
                    nc.sync.dma_start(out=smu[s + 1][SB:SB + 1, :],
                                      in_=ct[SB:SB + 1, :])
                else:
                    nc.sync.dma_start(out=xtail[127:128, :],
                                      in_=ct[SB:SB + 1, :])

                # ---------- d pass ----------
                esb = esb_pool.tile([128, CW], F16, tag="esb",
                                    name=f"esb{s}")
                esbs[s] = esb
                pmv = pmv_pool.tile([SB, 512], F32, tag="pmv", name=f"pmv{s}")
                for i in range(SB):
                    cols = slice(512 * i, 512 * i + 512)
                    pd = pd_pool.tile([128, 512], F32, tag="pd")
                    nc.tensor.matmul(pd[:, :], wsb["wd"][:, :],
                                     xsb[0:128, cols], start=True, stop=True)
                    # d overwrites the consumed x block in place (fp16)
                    nc.vector.tensor_copy(xsb[0:B, cols], pd[0:B, :])
                    nc.vector.tensor_mul(esb[0:B, cols], xsb[0:B, cols],
                                         xsb[0:B, cols])
                    nc.tensor.matmul(pmv[0:SB, :],
                                     wsb["wmv8"][:, SB * i:SB * i + SB],
                                     esb[0:B, cols],
                                     start=(i == 0), stop=(i == SB - 1))

                # ---------- var scan ----------
                nc.vector.tensor_copy(smv[s][0:SB, :], pmv[0:SB, :])
                pcv = pcv_pool.tile([SB + 1, 512], F32, tag="pcv",
                                    name=f"pcv{s}")
                nc.tensor.matmul(pcv[0:SB + 1, :], wsb["ls"][:, :],
                                 smv[s][0:SB + 1, :], start=True, stop=True)
                cvt = cvt_pool.tile([SB + 1, 512], F16, tag="cvt",
                                    name=f"cvt{s}")
                nc.vector.tensor_copy(cvt[0:SB + 1, :], pcv[0:SB + 1, :])
                nc.sync.dma_start(out=esb[127:128, 0:CW], in_=cvt[0:SB, :])
                if s < NSB - 1:
                    smv[s + 1] = smv_pool.tile([SB + 1, 512], F16, tag="smv",
                                               name=f"smv{s + 1}")
                    nc.sync.dma_start(out=smv[s + 1][SB:SB + 1, :],
                                      in_=cvt[SB:SB + 1, :])
                else:
                    nc.sync.dma_start(out=etail[127:128, :],
                                      in_=cvt[SB:SB + 1, :])

                # ---------- v pass ----------
                ysb = ysb_pool.tile([128, CW], F16, tag="ysb",
                                    name=f"ysb{s}")
                for i in range(SB):
                    cols = slice(512 * i, 512 * i + 512)
                    pv = pv_pool.tile([128, 512], F32, tag="pv")
                    nc.tensor.matmul(pv[:, :], wsb["tv"][:, :],
                                     esb[0:128, cols], start=True, stop=True)
                    rs = rs_pool.tile([128, 512], F16, tag="rs")
                    nc.scalar.activation(rs[0:B, :], pv[0:B, :],
                                         AF.Abs_reciprocal_sqrt,
                                         bias=eps_sb[0:B, :])
                    nc.vector.tensor_mul(ysb[0:B, cols], xsb[0:B, cols],
                                         rs[0:B, :])
                nc.sync.dma_start(out=y[128 * s:128 * s + B, :],
                                  in_=ysb[0:B, 0:CW])

            # ---------- tail block (64 rows) ----------
            pdt = pd_pool.tile([128, 512], F32, tag="pd", name="pdt")
            nc.tensor.matmul(pdt[:, :], wsb["wd"][:, :], xtail[0:128, :],
                             start=True, stop=True)
            nc.vector.tensor_copy(xtail[0:B, :], pdt[0:B, :])
            nc.vector.tensor_mul(etail[0:B, :], xtail[0:B, :], xtail[0:B, :])
            pvt = pv_pool.tile([128, 512], F32, tag="pv", name="pvt")
            nc.tensor.matmul(pvt[:, :], wsb["tv"][:, :], etail[0:128, :],
                             start=True, stop=True)
            rst = rs_pool.tile([128, 512], F16, tag="rs", name="rst")
            nc.scalar.activation(rst[0:B, :], pvt[0:B, :],
                                 AF.Abs_reciprocal_sqrt, bias=eps_sb[0:B, :])
            ytail = cpool.tile([128, 512], F16, tag="ytail", name="ytail")
            nc.vector.tensor_mul(ytail[0:B, :], xtail[0:B, :], rst[0:B, :])
            nc.sync.dma_start(out=y[8 * 128:8 * 128 + TAIL_ROWS, 0:512],
                              in_=ytail[0:TAIL_ROWS, :])

    nc.compile()
    return nc


_NC_CACHE = {}


def _get_nc():
    key = L_SHARD
    if key not in _NC_CACHE:
        _NC_CACHE[key] = _build_nc(key)
    return _NC_CACHE[key]


def kernel(x, mu0, var0, _want_time=False, _trace=False):
    x = np.asarray(x)
    mu0 = np.asarray(mu0).reshape(1, -1)
    var0 = np.asarray(var0).reshape(1, -1)
    assert x.shape == (N_ROWS, L_FULL), x.shape

    nc = _get_nc()
    xf16 = x.astype(np.float16)  # [8192, 4096]
    in_maps = []
    for c in range(N_CORES):
        sl = slice(c * L_SHARD, (c + 1) * L_SHARD)
        xc = xf16[:, sl]  # [8192, 512]
        xdev = np.zeros((9 * 128, 8 * 512), dtype=np.float16)
        # slab s row k col 512i+f = xc[127*(8s+i)+k, f]
        m = xc[:64 * B].reshape(8, SB, B, L_SHARD).transpose(0, 2, 1, 3)
        xdev.reshape(9, 128, 8 * 512)[:8, :B, :] = m.reshape(8, B, 8 * 512)
        xdev[8 * 128:8 * 128 + TAIL_ROWS, 0:512] = xc[64 * B:]
        in_maps.append({
            "x": xdev,
            "mu0": np.ascontiguousarray(mu0[:, sl]).astype(np.float16),
            "var0": np.ascontiguousarray(var0[:, sl]).astype(np.float16),
            **_WEIGHTS,
        })

    exec_ns = None
    if _trace:
        orig_upload = bass_utils.upload_artifacts
        bass_utils.upload_artifacts = lambda tmpdir: "(skipped)"
        try:
            res = bass_utils.run_bass_kernel_spmd(
                nc, in_maps, list(range(N_CORES)), trace=True
            )
            exec_ns = res.exec_time_ns
        finally:
            bass_utils.upload_artifacts = orig_upload
    else:
        res = bass_utils.run_bass_kernel_spmd(nc, in_maps, list(range(N_CORES)))

    out = np.empty((N_ROWS, L_FULL), dtype=np.float32)
    for c in range(N_CORES):
        sl = slice(c * L_SHARD, (c + 1) * L_SHARD)
        ydev = res.results[c]["y"]  # [1152, 4096] f16
        m = ydev.reshape(9, 128, SB, L_SHARD)[:8, :B]  # [s, k, i, f]
        out[:64 * B, sl] = m.transpose(0, 2, 1, 3).reshape(64 * B, L_SHARD)
        out[64 * B:, sl] = ydev[8 * 128:8 * 128 + TAIL_ROWS, 0:512]
    if _want_time:
        return out, exec_ns
    return out


# revision 15
# speedup vs baseline: 3.0813x; 2.6853x over previous
"""Online Normalization forward (nn_Norm1d) on 8 Trainium2 NeuronCores.

Reference recurrence over the batch dim t (per feature, sequential):
    d_t   = x_t - mu_t
    y_t   = d_t / sqrt(var_t + eps)
    mu_{t+1}  = mu_t + (1-a)*d_t
    var_{t+1} = a*var_t + a*(1-a)*d_t^2

Sharding: tensor-parallel over the feature dim L (4096 -> 8 x 512); each
feature's scan over N=8192 is independent, so no cross-core communication.

Kernel structure (per core, 512 features):
  - fp16 I/O: host casts x to fp16 (halves HBM reads), y stored fp16 and
    upcast on host (halves HBM writes).
  - 127-step blocks; time lives on SBUF partitions 0..126, the running
    carry (mu or var) rides partition 127 of the same moving tile, so ONE
    matmul per block per path computes all 127 outputs AND the next carry.
  - The serial block-to-block carry chain is broken with a block-level
    scan: per-block carry increments m(b) are computed by independent
    "m-matmuls" that accumulate into one PSUM tile (stationary with only
    column b-mod-8 nonzero), then one small scan-matmul (lower-triangular
    in A = a^127) produces all 8 carries of a super-block at once, and a
    single SBUF->SBUF DMA scatters them into partition 127 of the x/e
    tiles.  Same machinery twice: mu-path (from x) and var-path (from
    e = d^2, which only exists after the d-pass -> the v-pass trails one
    super-block behind; the Tile scheduler overlaps them).
  - Elementwise work: dcopy (PSUM->SBUF fp16, overwrites the consumed x
    block in place), square, y-mul on the vector engine (2x fp16 modes);
    rsqrt (Abs_reciprocal_sqrt table) on the scalar engine.
  - DMA dispatch spread: x-loads on gpsimd (SWDGE), y-stores and scatters
    on sync (HWDGE), so neither compute engine pays dispatch time.
"""

import sys

for _p in ("/opt/trn_rl_repo", "/root/.axon_site/_ro/trn_rl_repo"):
    if _p not in sys.path:
        sys.path.append(_p)

import numpy as np

import concourse.bacc as bacc
import concourse.mybir as mybir
from concourse.tile import TileContext
from concourse import bass_utils

N_ROWS = 8192
L_FULL = 4096
N_CORES = 8
L_SHARD = L_FULL // N_CORES

AFWD = 0.999
EPS = 1e-05
B = 127          # time steps per block (partition 127 = carry row)
NBLK = 65        # 64 full blocks + 64-row tail
SB = 8           # blocks per super-block
NSB = 8          # super-blocks covering blocks 0..63
TAIL_ROWS = N_ROWS - 64 * B  # 64

F32 = mybir.dt.float32
F16 = mybir.dt.float16
AF = mybir.ActivationFunctionType


def _build_weights():
    A = AFWD
    # WD [128,128]: [j,k] = coeff of moving row j for output col k.
    WD = np.zeros((128, 128))
    for k in range(B):
        WD[k, k] = 1.0
        for j in range(k):
            WD[j, k] = -(1 - A) * A ** (k - 1 - j)
        WD[127, k] = -(A ** k)
    for j in range(B):
        WD[j, 127] = (1 - A) * A ** (126 - j)
    WD[127, 127] = A ** 127

    # TV [128,128]: v^(k) = a^k*nu + (1-a) sum_{j<k} a^(k-j) e_j
    TV = np.zeros((128, 128))
    for k in range(B):
        for j in range(k):
            TV[j, k] = (1 - A) * A ** (k - j)
        TV[127, k] = A ** k
    for j in range(B):
        TV[j, 127] = (1 - A) * A ** (127 - j)
    TV[127, 127] = A ** 127

    # WM8 [127, 64]: 8 stacked [127,8] stationaries; slot i has only col i
    # nonzero = per-block mu increment coefficients.
    WM8 = np.zeros((B, SB * SB))
    WMv8 = np.zeros((B, SB * SB))
    for i in range(SB):
        for j in range(B):
            WM8[j, SB * i + i] = (1 - A) * A ** (126 - j)
            WMv8[j, SB * i + i] = (1 - A) * A ** (127 - j)

    # LS [9,9]: block-level scan in Abig = a^127.
    Abig = A ** B
    LS = np.zeros((SB + 1, SB + 1))
    for k in range(SB + 1):
        for i in range(min(k, SB)):
            LS[i, k] = Abig ** (k - 1 - i)
        LS[SB, k] = Abig ** k
    return {"wd": WD, "tv": TV, "wm8": WM8, "wmv8": WMv8, "ls": LS}


_WEIGHTS = {k: np.ascontiguousarray(v.astype(np.float16))
            for k, v in _build_weights().items()}


def _build_nc(l_cols: int):
    # x/y DRAM layout: 9 slabs of 128 rows x 4096 cols (fp16).  Slab s<8:
    # row k, col 512*i+f = x[127*(8s+i)+k, f] -- so one contiguous
    # [127, 4096] DMA loads a whole super-block.  Slab 8 (tail): cols 0:512
    # hold x rows 8128..8191 in rows 0..63, zeros elsewhere (host-built).
    nc = bacc.Bacc()
    x = nc.declare_dram_parameter("x", [9 * 128, 8 * 512], F16,
                                  isOutput=False)
    mu0 = nc.declare_dram_parameter("mu0", [1, l_cols], F16, isOutput=False)
    var0 = nc.declare_dram_parameter("var0", [1, l_cols], F16, isOutput=False)
    wts = {
        name: nc.declare_dram_parameter(name, list(w.shape), F16,
                                        isOutput=False)
        for name, w in _WEIGHTS.items()
    }
    y = nc.declare_dram_parameter("y", [9 * 128, 8 * 512], F16,
                                  isOutput=True)

    CW = SB * 512  # columns per super-block tile (4096)

    with TileContext(nc) as tc:
        with (
            tc.tile_pool(name="consts", bufs=1) as cpool,
            tc.tile_pool(name="xsb", bufs=3) as xsb_pool,
            tc.tile_pool(name="esb", bufs=2) as esb_pool,
            tc.tile_pool(name="ysb", bufs=2) as ysb_pool,
            tc.tile_pool(name="rs", bufs=3) as rs_pool,
            tc.tile_pool(name="smov", bufs=2) as smu_pool,
            tc.tile_pool(name="smovv", bufs=2) as smv_pool,
            tc.tile_pool(name="ct", bufs=2) as ct_pool,
            tc.tile_pool(name="cvt", bufs=2) as cvt_pool,
            tc.tile_pool(name="pd", bufs=2, space="PSUM") as pd_pool,
            tc.tile_pool(name="pv", bufs=2, space="PSUM") as pv_pool,
            tc.tile_pool(name="pm", bufs=1, space="PSUM") as pm_pool,
            tc.tile_pool(name="pmv", bufs=1, space="PSUM") as pmv_pool,
            tc.tile_pool(name="pc", bufs=1, space="PSUM") as pc_pool,
            tc.tile_pool(name="pcv", bufs=1, space="PSUM") as pcv_pool,
        ):
            wsb = {}
            for name, w in _WEIGHTS.items():
                wsb[name] = cpool.tile(list(w.shape), F16, tag=name,
                                       name=f"w_{name}")
                nc.sync.dma_start(out=wsb[name][:, :], in_=wts[name][:, :])
            eps_sb = cpool.tile([128, 1], F32, tag="eps", name="eps_sb")
            nc.vector.memset(eps_sb[:, :], EPS)

            # tail block tiles (block 64: rows 8128..8191, 64 valid rows;
            # host zero-pads rows 64..126 in the DRAM slab)
            xtail = cpool.tile([128, 512], F16, tag="xtail", name="xtail")
            etail = cpool.tile([128, 512], F16, tag="etail", name="etail")
            nc.gpsimd.dma_start(out=xtail[0:128, :],
                                in_=x[8 * 128:8 * 128 + 128, 0:512])

            smu = [None] * (NSB + 1)
            smv = [None] * (NSB + 1)
            smu[0] = smu_pool.tile([SB + 1, 512], F16, tag="smu", name="smu0")
            smv[0] = smv_pool.tile([SB + 1, 512], F16, tag="smv", name="smv0")
            nc.sync.dma_start(out=smu[0][SB:SB + 1, :], in_=mu0[:, :])
            nc.sync.dma_start(out=smv[0][SB:SB + 1, :], in_=var0[:, :])

            xsbs = [None] * NSB
            esbs = [None] * NSB

            for s in range(NSB):
                # ---------- mu prologue ----------
                xsb = xsb_pool.tile([128, CW], F16, tag="xsb",
                                    name=f"xsb{s}")
                xsbs[s] = xsb
                nc.gpsimd.dma_start(out=xsb[0:128, :],
                                    in_=x[128 * s:128 * s + 128, :])
                pm = pm_pool.tile([SB, 512], F32, tag="pm", name=f"pm{s}")
                for i in range(SB):
                    nc.tensor.matmul(pm[0:SB, :],
                                     wsb["wm8"][:, SB * i:SB * i + SB],
                                     xsb[0:B, 512 * i:512 * i + 512],
                                     start=(i == 0), stop=(i == SB - 1))
                nc.vector.tensor_copy(smu[s][0:SB, :], pm[0:SB, :])
                pc = pc_pool.tile([SB + 1, 512], F32, tag="pc", name=f"pc{s}")
                nc.tensor.matmul(pc[0:SB + 1, :], wsb["ls"][:, :],
                                 smu[s][0:SB + 1, :], start=True, stop=True)
                ct = ct_pool.tile([SB + 1, 512], F16, tag="ct", name=f"ct{s}")
                nc.vector.tensor_copy(ct[0:SB + 1, :], pc[0:SB + 1, :])
                # scatter mu carries into partition 127 of xsb
                nc.sync.dma_start(out=xsb[127:128, 0:CW], in_=ct[0:SB, :])
                if s < NSB - 1:
                    smu[s + 1] = smu_pool.tile([SB + 1, 512], F16, tag="smu",
                                               name=f"smu{s + 1}")
                    nc.sync.dma_start(out=smu[s + 1][SB:SB + 1, :],
                                      in_=ct[SB:SB + 1, :])
                else:
                    nc.sync.dma_start(out=xtail[127:128, :],
                                      in_=ct[SB:SB + 1, :])

                # ---------- d pass ----------
                esb = esb_pool.tile([128, CW], F16, tag="esb",
                                    name=f"esb{s}")
                esbs[s] = esb
                pmv = pmv_pool.tile([SB, 512], F32, tag="pmv", name=f"pmv{s}")
                for i in range(SB):
                    cols = slice(512 * i, 512 * i + 512)
                    pd = pd_pool.tile([128, 512], F32, tag="pd")
                    nc.tensor.matmul(pd[:, :], wsb["wd"][:, :],
                                     xsb[0:128, cols], start=True, stop=True)
                    # d overwrites the consumed x block in place (fp16)
                    nc.vector.tensor_copy(xsb[0:128, cols],
                                          pd[0:128, :])
                    nc.vector.tensor_mul(esb[0:128, cols], xsb[0:128, cols],
                                         xsb[0:128, cols])
                    nc.tensor.matmul(pmv[0:SB, :],
                                     wsb["wmv8"][:, SB * i:SB * i + SB],
                                     esb[0:B, cols],
                                     start=(i == 0), stop=(i == SB - 1))

                # ---------- var scan ----------
                nc.vector.tensor_copy(smv[s][0:SB, :], pmv[0:SB, :])
                pcv = pcv_pool.tile([SB + 1, 512], F32, tag="pcv",
                                    name=f"pcv{s}")
                nc.tensor.matmul(pcv[0:SB + 1, :], wsb["ls"][:, :],
                                 smv[s][0:SB + 1, :], start=True, stop=True)
                cvt = cvt_pool.tile([SB + 1, 512], F16, tag="cvt",
                                    name=f"cvt{s}")
                nc.vector.tensor_copy(cvt[0:SB + 1, :], pcv[0:SB + 1, :])
                nc.sync.dma_start(out=esb[127:128, 0:CW], in_=cvt[0:SB, :])
                if s < NSB - 1:
                    smv[s + 1] = smv_pool.tile([SB + 1, 512], F16, tag="smv",
                                               name=f"smv{s + 1}")
                    nc.sync.dma_start(out=smv[s + 1][SB:SB + 1, :],
                                      in_=cvt[SB:SB + 1, :])
                else:
                    nc.sync.dma_start(out=etail[127:128, :],
                                      in_=cvt[SB:SB + 1, :])

                # ---------- v pass ----------
                ysb = ysb_pool.tile([128, CW], F16, tag="ysb",
                                    name=f"ysb{s}")
                for i in range(SB):
                    cols = slice(512 * i, 512 * i + 512)
                    pv = pv_pool.tile([128, 512], F32, tag="pv")
                    nc.tensor.matmul(pv[:, :], wsb["tv"][:, :],
                                     esb[0:128, cols], start=True, stop=True)
                    rs = rs_pool.tile([128, 512], F16, tag="rs")
                    nc.scalar.activation(rs[0:128, :], pv[0:128, :],
                                         AF.Abs_reciprocal_sqrt,
                                         bias=eps_sb[0:128, :])
                    nc.vector.tensor_mul(ysb[0:128, cols], xsb[0:128, cols],
                                         rs[0:128, :])
                nc.sync.dma_start(out=y[128 * s:128 * s + 128, :],
                                  in_=ysb[0:128, 0:CW])

            # ---------- tail block (64 rows) ----------
            pdt = pd_pool.tile([128, 512], F32, tag="pd", name="pdt")
            nc.tensor.matmul(pdt[:, :], wsb["wd"][:, :], xtail[0:128, :],
                             start=True, stop=True)
            nc.vector.tensor_copy(xtail[0:128, :], pdt[0:128, :])
            # row 127 of etail holds the var carry written at s=7 -- the
            # square must not clobber it
            nc.vector.tensor_mul(etail[0:B, :], xtail[0:B, :],
                                 xtail[0:B, :])
            pvt = pv_pool.tile([128, 512], F32, tag="pv", name="pvt")
            nc.tensor.matmul(pvt[:, :], wsb["tv"][:, :], etail[0:128, :],
                             start=True, stop=True)
            rst = rs_pool.tile([128, 512], F16, tag="rs", name="rst")
            nc.scalar.activation(rst[0:128, :], pvt[0:128, :],
                                 AF.Abs_reciprocal_sqrt,
                                 bias=eps_sb[0:128, :])
            ytail = cpool.tile([128, 512], F16, tag="ytail", name="ytail")
            nc.vector.tensor_mul(ytail[0:128, :], xtail[0:128, :],
                                 rst[0:128, :])
            nc.sync.dma_start(out=y[8 * 128:9 * 128, 0:512],
                              in_=ytail[0:128, :])

    nc.compile()
    return nc


_NC_CACHE = {}


def _get_nc():
    key = L_SHARD
    if key not in _NC_CACHE:
        _NC_CACHE[key] = _build_nc(key)
    return _NC_CACHE[key]


def kernel(x, mu0, var0, _want_time=False, _trace=False):
    x = np.asarray(x)
    mu0 = np.asarray(mu0).reshape(1, -1)
    var0 = np.asarray(var0).reshape(1, -1)
    assert x.shape == (N_ROWS, L_FULL), x.shape

    nc = _get_nc()
    xf16 = x.astype(np.float16)  # [8192, 4096]
    in_maps = []
    for c in range(N_CORES):
        sl = slice(c * L_SHARD, (c + 1) * L_SHARD)
        xc = xf16[:, sl]  # [8192, 512]
        xdev = np.zeros((9 * 128, 8 * 512), dtype=np.float16)
        # slab s row k col 512i+f = xc[127*(8s+i)+k, f]
        m = xc[:64 * B].reshape(8, SB, B, L_SHARD).transpose(0, 2, 1, 3)
        xdev.reshape(9, 128, 8 * 512)[:8, :B, :] = m.reshape(8, B, 8 * 512)
        xdev[8 * 128:8 * 128 + TAIL_ROWS, 0:512] = xc[64 * B:]
        in_maps.append({
            "x": xdev,
            "mu0": np.ascontiguousarray(mu0[:, sl]).astype(np.float16),
            "var0": np.ascontiguousarray(var0[:, sl]).astype(np.float16),
            **_WEIGHTS,
        })

    exec_ns = None
    if _trace:
        orig_upload = bass_utils.upload_artifacts
        bass_utils.upload_artifacts = lambda tmpdir: "(skipped)"
        try:
            res = bass_utils.run_bass_kernel_spmd(
                nc, in_maps, list(range(N_CORES)), trace=True
            )
            exec_ns = res.exec_time_ns
        finally:
            bass_utils.upload_artifacts = orig_upload
    else:
        res = bass_utils.run_bass_kernel_spmd(nc, in_maps, list(range(N_CORES)))

    out = np.empty((N_ROWS, L_FULL), dtype=np.float32)
    for c in range(N_CORES):
        sl = slice(c * L_SHARD, (c + 1) * L_SHARD)
        ydev = res.results[c]["y"]  # [1152, 4096] f16
        m = ydev.reshape(9, 128, SB, L_SHARD)[:8, :B]  # [s, k, i, f]
        out[:64 * B, sl] = m.transpose(0, 2, 1, 3).reshape(64 * B, L_SHARD)
        out[64 * B:, sl] = ydev[8 * 128:8 * 128 + TAIL_ROWS, 0:512]
    if _want_time:
        return out, exec_ns
    return out


# revision 16
# speedup vs baseline: 3.4804x; 1.1295x over previous
"""Online Normalization forward (nn_Norm1d) on 8 Trainium2 NeuronCores.

Reference recurrence over the batch dim t (per feature, sequential):
    d_t   = x_t - mu_t
    y_t   = d_t / sqrt(var_t + eps)
    mu_{t+1}  = mu_t + (1-a)*d_t
    var_{t+1} = a*var_t + a*(1-a)*d_t^2

Sharding: tensor-parallel over the feature dim L (4096 -> 8 x 512); each
feature's scan over N=8192 is independent -> no cross-core communication.

Kernel structure (per core, 512 features):
  - fp16 I/O, host-side cast + block-slab relayout: x and y live in DRAM as
    5 slabs of [128, 8192]: slab s row k col 512*i+f = x[127*(16s+i)+k, f],
    so every bulk DMA is a full-128-partition contiguous transfer (the
    16-engine descriptor spray path).
  - 127-step blocks: time on partitions 0..126, the running carry (mu or
    var) rides partition 127 of the same moving tile; one [128,128]
    stationary computes all 127 d's (or var's) of a block in one matmul.
  - Block-to-block carries come from a fused block-level scan: per block,
    one extra matmul with a scan-weighted stationary (cols = all 16 carry
    outputs of the super-block + next carry at col 0) accumulates into a
    [17,512] PSUM tile; a K=1 inject matmul adds the incoming carry; one
    DVE copy + one SBUF->SBUF scatter DMA plant the carries into partition
    127 of the x/e tiles.  Same machinery for mu (from x) and var (from
    e=d^2; the var pass trails one super-block so nothing ever waits).
  - Elementwise ops run pair-wide (FD=1024 over two adjacent PSUM banks):
    dcopy (d overwrites the consumed x block), square, y-mul on vector;
    rsqrt (Abs_reciprocal_sqrt) on scalar.
  - Software pipelining: the loop issues prologue(s+1), d-pass(s),
    v-pass(s-1), so every engine's FIFO queue only sees ready work.
"""

import sys

for _p in ("/opt/trn_rl_repo", "/root/.axon_site/_ro/trn_rl_repo"):
    if _p not in sys.path:
        sys.path.append(_p)

import numpy as np

import concourse.bacc as bacc
import concourse.mybir as mybir
from concourse.tile import TileContext
from concourse import bass_utils

N_ROWS = 8192
L_FULL = 4096
N_CORES = 8
L_SHARD = L_FULL // N_CORES

AFWD = 0.999
EPS = 1e-05
B = 127           # time steps per block (partition 127 = carry row)
SB = 16           # blocks per super-block
NSB = 4           # super-blocks covering blocks 0..63
TAIL_ROWS = N_ROWS - 64 * B  # 64
CW = SB * 512     # 8192 cols per slab

F32 = mybir.dt.float32
F16 = mybir.dt.float16
AF = mybir.ActivationFunctionType


def _build_weights():
    A = AFWD
    Abig = A ** B
    # WD [128,128]: [j,k] = coeff of moving row j for output col k.
    WD = np.zeros((128, 128))
    for k in range(B):
        WD[k, k] = 1.0
        for j in range(k):
            WD[j, k] = -(1 - A) * A ** (k - 1 - j)
        WD[127, k] = -(A ** k)
    for j in range(B):
        WD[j, 127] = (1 - A) * A ** (126 - j)
    WD[127, 127] = Abig

    # TV [128,128]: v^(k) = a^k*nu + (1-a) sum_{j<k} a^(k-j) e_j
    TV = np.zeros((128, 128))
    for k in range(B):
        for j in range(k):
            TV[j, k] = (1 - A) * A ** (k - j)
        TV[127, k] = A ** k
    for j in range(B):
        TV[j, 127] = (1 - A) * A ** (127 - j)
    TV[127, 127] = Abig

    # Per-block mu / var increment row vectors.
    wm = np.array([(1 - A) * A ** (126 - j) for j in range(B)])
    wmv = np.array([(1 - A) * A ** (127 - j) for j in range(B)])

    # Fused scan stationaries WSM/WSV [127, 16*17]: slot i (block i within
    # the super-block) is a [127,17] stationary.  Output rows of the
    # accumulated [17,512] PSUM tile:
    #   row 0      = next carry  = Abig^16 c + sum_i Abig^(15-i) m(i)
    #   row r=1..16: carry of block 16s+(r-1)
    #              = Abig^(r-1) c + sum_{i<=r-2} Abig^(r-2-i) m(i)
    WSM = np.zeros((B, SB * (SB + 1)))
    WSV = np.zeros((B, SB * (SB + 1)))
    for i in range(SB):
        base = (SB + 1) * i
        WSM[:, base + 0] = Abig ** (SB - 1 - i) * wm
        WSV[:, base + 0] = Abig ** (SB - 1 - i) * wmv
        for r in range(i + 2, SB + 1):
            WSM[:, base + r] = Abig ** (r - 2 - i) * wm
            WSV[:, base + r] = Abig ** (r - 2 - i) * wmv

    # Carry inject stationary IC [1, 17].
    IC = np.zeros((1, SB + 1))
    IC[0, 0] = Abig ** SB
    for r in range(1, SB + 1):
        IC[0, r] = Abig ** (r - 1)
    return {"wd": WD, "tv": TV, "wsm": WSM, "wsv": WSV, "ic": IC}


_WEIGHTS = {k: np.ascontiguousarray(v.astype(np.float16))
            for k, v in _build_weights().items()}


def _build_nc(l_cols: int):
    nc = bacc.Bacc()
    x = nc.declare_dram_parameter("x", [5 * 128, CW], F16, isOutput=False)
    mu0 = nc.declare_dram_parameter("mu0", [1, l_cols], F16, isOutput=False)
    var0 = nc.declare_dram_parameter("var0", [1, l_cols], F16, isOutput=False)
    wts = {
        name: nc.declare_dram_parameter(name, list(w.shape), F16,
                                        isOutput=False)
        for name, w in _WEIGHTS.items()
    }
    y = nc.declare_dram_parameter("y", [5 * 128, CW], F16, isOutput=True)

    with TileContext(nc) as tc:
        with (
            tc.tile_pool(name="consts", bufs=1) as cpool,
            tc.tile_pool(name="xsb", bufs=3) as xsb_pool,
            tc.tile_pool(name="esb", bufs=3) as esb_pool,
            tc.tile_pool(name="ysb", bufs=2) as ysb_pool,
            tc.tile_pool(name="rs", bufs=3) as rs_pool,
            tc.tile_pool(name="ct", bufs=2) as ct_pool,
            tc.tile_pool(name="cvt", bufs=2) as cvt_pool,
            tc.tile_pool(name="pd", bufs=2, space="PSUM") as pd_pool,
            tc.tile_pool(name="pv", bufs=1, space="PSUM") as pv_pool,
            tc.tile_pool(name="pc", bufs=1, space="PSUM") as pc_pool,
            tc.tile_pool(name="pcv", bufs=1, space="PSUM") as pcv_pool,
        ):
            wsb = {}
            for name, w in _WEIGHTS.items():
                wsb[name] = cpool.tile(list(w.shape), F16, tag=name,
                                       name=f"w_{name}")
                nc.sync.dma_start(out=wsb[name][:, :], in_=wts[name][:, :])
            eps_sb = cpool.tile([128, 1], F32, tag="eps", name="eps_sb")
            nc.vector.memset(eps_sb[:, :], EPS)

            # initial carries (partition 0 of [1,512] tiles)
            cm0 = cpool.tile([1, 512], F16, tag="cm0", name="cm0")
            cv0 = cpool.tile([1, 512], F16, tag="cv0", name="cv0")
            nc.sync.dma_start(out=cm0[0:1, :], in_=mu0[:, :])
            nc.sync.dma_start(out=cv0[0:1, :], in_=var0[:, :])

            # tail block tiles (block 64: rows 8128..8191; host zero-pads)
            xtail = cpool.tile([128, 512], F16, tag="xtail", name="xtail")
            etail = cpool.tile([128, 512], F16, tag="etail", name="etail")
            nc.gpsimd.dma_start(out=xtail[0:128, :],
                                in_=x[4 * 128:5 * 128, 0:512])

            xsbs = [None] * NSB
            esbs = [None] * NSB
            cts = [None] * NSB
            cvts = [None] * NSB

            def prologue(s):
                xsb = xsb_pool.tile([128, CW], F16, tag="xsb",
                                    name=f"xsb{s}")
                xsbs[s] = xsb
                nc.gpsimd.dma_start(out=xsb[0:128, :],
                                    in_=x[128 * s:128 * s + 128, :])
                pc = pc_pool.tile([SB + 1, 512], F32, tag="pc",
                                  name=f"pc{s}")
                for i in range(SB):
                    base = (SB + 1) * i
                    nc.tensor.matmul(pc[0:SB + 1, :],
                                     wsb["wsm"][:, base:base + SB + 1],
                                     xsb[0:B, 512 * i:512 * i + 512],
                                     start=(i == 0), stop=False)
                carry = cm0 if s == 0 else cts[s - 1]
                nc.tensor.matmul(pc[0:SB + 1, :], wsb["ic"][:, :],
                                 carry[0:1, :], start=False, stop=True)
                ct = ct_pool.tile([SB + 1, 512], F16, tag="ct",
                                  name=f"ct{s}")
                cts[s] = ct
                nc.vector.tensor_copy(ct[0:SB + 1, :], pc[0:SB + 1, :])
                nc.sync.dma_start(out=xsb[127:128, 0:CW], in_=ct[1:SB + 1, :])
                if s == NSB - 1:
                    nc.sync.dma_start(out=xtail[127:128, :], in_=ct[0:1, :])

            def dpass(s):
                xsb = xsbs[s]
                esb = esb_pool.tile([128, CW], F16, tag="esb",
                                    name=f"esb{s}")
                esbs[s] = esb
                pcv = pcv_pool.tile([SB + 1, 512], F32, tag="pcv",
                                    name=f"pcv{s}")
                for q in range(SB // 2):
                    c0 = 1024 * q
                    pd = pd_pool.tile([128, 1024], F32, tag="pd")
                    nc.tensor.matmul(pd[:, 0:512], wsb["wd"][:, :],
                                     xsb[0:128, c0:c0 + 512],
                                     start=True, stop=True)
                    nc.tensor.matmul(pd[:, 512:1024], wsb["wd"][:, :],
                                     xsb[0:128, c0 + 512:c0 + 1024],
                                     start=True, stop=True)
                    nc.vector.tensor_copy(xsb[0:128, c0:c0 + 1024],
                                          pd[0:128, :])
                    nc.vector.tensor_mul(esb[0:128, c0:c0 + 1024],
                                         xsb[0:128, c0:c0 + 1024],
                                         xsb[0:128, c0:c0 + 1024])
                    for h in range(2):
                        i = 2 * q + h
                        base = (SB + 1) * i
                        nc.tensor.matmul(pcv[0:SB + 1, :],
                                         wsb["wsv"][:, base:base + SB + 1],
                                         esb[0:B, 512 * i:512 * i + 512],
                                         start=(i == 0), stop=False)
                carry = cv0 if s == 0 else cvts[s - 1]
                nc.tensor.matmul(pcv[0:SB + 1, :], wsb["ic"][:, :],
                                 carry[0:1, :], start=False, stop=True)
                cvt = cvt_pool.tile([SB + 1, 512], F16, tag="cvt",
                                    name=f"cvt{s}")
                cvts[s] = cvt
                nc.vector.tensor_copy(cvt[0:SB + 1, :], pcv[0:SB + 1, :])
                nc.sync.dma_start(out=esb[127:128, 0:CW],
                                  in_=cvt[1:SB + 1, :])
                if s == NSB - 1:
                    nc.sync.dma_start(out=etail[127:128, :], in_=cvt[0:1, :])

            def vpass(s):
                xsb, esb = xsbs[s], esbs[s]
                ysb = ysb_pool.tile([128, CW], F16, tag="ysb",
                                    name=f"ysb{s}")
                for q in range(SB // 2):
                    c0 = 1024 * q
                    pv = pv_pool.tile([128, 1024], F32, tag="pv")
                    nc.tensor.matmul(pv[:, 0:512], wsb["tv"][:, :],
                                     esb[0:128, c0:c0 + 512],
                                     start=True, stop=True)
                    nc.tensor.matmul(pv[:, 512:1024], wsb["tv"][:, :],
                                     esb[0:128, c0 + 512:c0 + 1024],
                                     start=True, stop=True)
                    rs = rs_pool.tile([128, 1024], F16, tag="rs")
                    nc.scalar.activation(rs[0:128, :], pv[0:128, :],
                                         AF.Abs_reciprocal_sqrt,
                                         bias=eps_sb[0:128, :])
                    nc.vector.tensor_mul(ysb[0:128, c0:c0 + 1024],
                                         xsb[0:128, c0:c0 + 1024],
                                         rs[0:128, :])
                nc.sync.dma_start(out=y[128 * s:128 * s + 128, :],
                                  in_=ysb[0:128, 0:CW])

            prologue(0)
            for s in range(NSB):
                if s + 1 < NSB:
                    prologue(s + 1)
                dpass(s)
                if s >= 1:
                    vpass(s - 1)
            vpass(NSB - 1)

            # ---------- tail block (64 rows) ----------
            pdt = pd_pool.tile([128, 1024], F32, tag="pd", name="pdt")
            nc.tensor.matmul(pdt[:, 0:512], wsb["wd"][:, :], xtail[0:128, :],
                             start=True, stop=True)
            nc.vector.tensor_copy(xtail[0:128, :], pdt[0:128, 0:512])
            # row 127 of etail holds the var carry -- don't clobber it
            nc.vector.tensor_mul(etail[0:B, :], xtail[0:B, :], xtail[0:B, :])
            pvt = pv_pool.tile([128, 1024], F32, tag="pv", name="pvt")
            nc.tensor.matmul(pvt[:, 0:512], wsb["tv"][:, :], etail[0:128, :],
                             start=True, stop=True)
            rst = rs_pool.tile([128, 1024], F16, tag="rs", name="rst")
            nc.scalar.activation(rst[0:128, 0:512], pvt[0:128, 0:512],
                                 AF.Abs_reciprocal_sqrt,
                                 bias=eps_sb[0:128, :])
            ytail = cpool.tile([128, 512], F16, tag="ytail", name="ytail")
            nc.vector.tensor_mul(ytail[0:128, :], xtail[0:128, :],
                                 rst[0:128, 0:512])
            nc.sync.dma_start(out=y[4 * 128:5 * 128, 0:512],
                              in_=ytail[0:128, :])

    nc.compile()
    return nc


_NC_CACHE = {}


def _get_nc():
    key = L_SHARD
    if key not in _NC_CACHE:
        _NC_CACHE[key] = _build_nc(key)
    return _NC_CACHE[key]


def kernel(x, mu0, var0, _want_time=False, _trace=False):
    x = np.asarray(x)
    mu0 = np.asarray(mu0).reshape(1, -1)
    var0 = np.asarray(var0).reshape(1, -1)
    assert x.shape == (N_ROWS, L_FULL), x.shape

    nc = _get_nc()
    xf16 = x.astype(np.float16)  # [8192, 4096]
    in_maps = []
    for c in range(N_CORES):
        sl = slice(c * L_SHARD, (c + 1) * L_SHARD)
        xc = xf16[:, sl]  # [8192, 512]
        xdev = np.zeros((5 * 128, CW), dtype=np.float16)
        # slab s row k col 512i+f = xc[127*(16s+i)+k, f]
        m = xc[:64 * B].reshape(NSB, SB, B, L_SHARD).transpose(0, 2, 1, 3)
        xdev.reshape(5, 128, CW)[:NSB, :B, :] = m.reshape(NSB, B, CW)
        xdev[4 * 128:4 * 128 + TAIL_ROWS, 0:512] = xc[64 * B:]
        in_maps.append({
            "x": xdev,
            "mu0": np.ascontiguousarray(mu0[:, sl]).astype(np.float16),
            "var0": np.ascontiguousarray(var0[:, sl]).astype(np.float16),
            **_WEIGHTS,
        })

    exec_ns = None
    if _trace:
        orig_upload = bass_utils.upload_artifacts
        bass_utils.upload_artifacts = lambda tmpdir: "(skipped)"
        try:
            res = bass_utils.run_bass_kernel_spmd(
                nc, in_maps, list(range(N_CORES)), trace=True
            )
            exec_ns = res.exec_time_ns
        finally:
            bass_utils.upload_artifacts = orig_upload
    else:
        res = bass_utils.run_bass_kernel_spmd(nc, in_maps, list(range(N_CORES)))

    out = np.empty((N_ROWS, L_FULL), dtype=np.float32)
    for c in range(N_CORES):
        sl = slice(c * L_SHARD, (c + 1) * L_SHARD)
        ydev = res.results[c]["y"]  # [640, 8192] f16
        m = ydev.reshape(5, 128, SB, L_SHARD)[:NSB, :B]  # [s, k, i, f]
        out[:64 * B, sl] = m.transpose(0, 2, 1, 3).reshape(64 * B, L_SHARD)
        out[64 * B:, sl] = ydev[4 * 128:4 * 128 + TAIL_ROWS, 0:512]
    if _want_time:
        return out, exec_ns
    return out


# revision 23
# speedup vs baseline: 3.8178x; 1.0969x over previous
"""Online Normalization forward (nn_Norm1d) on 8 Trainium2 NeuronCores.

Reference recurrence over the batch dim t (per feature, sequential):
    d_t   = x_t - mu_t
    y_t   = d_t / sqrt(var_t + eps)
    mu_{t+1}  = mu_t + (1-a)*d_t
    var_{t+1} = a*var_t + a*(1-a)*d_t^2

Sharding: tensor-parallel over the feature dim L (4096 -> 8 x 512); each
feature's scan over N=8192 is independent -> no cross-core communication.

Kernel structure (per core, 512 features):
  - fp16 I/O, host-side cast + block-slab relayout: x and y live in DRAM as
    5 slabs of [128, 8192]: slab s row k col 512*i+f = x[127*(16s+i)+k, f],
    so every bulk DMA is a full-128-partition contiguous transfer (the
    16-engine descriptor spray path).
  - 127-step blocks: time on partitions 0..126, the running carry (mu or
    var) rides partition 127 of the same moving tile; one [128,128]
    stationary computes all 127 d's (or var's) of a block in one matmul.
  - Block-to-block carries come from a fused block-level scan: per block,
    one extra matmul with a scan-weighted stationary (cols = all 16 carry
    outputs of the super-block + next carry at col 0) accumulates into a
    [17,512] PSUM tile; a K=1 inject matmul adds the incoming carry; one
    DVE copy + one SBUF->SBUF scatter DMA plant the carries into partition
    127 of the x/e tiles.  Same machinery for mu (from x) and var (from
    e=d^2; the var pass trails one super-block so nothing ever waits).
  - Elementwise ops run pair-wide (FD=1024 over two adjacent PSUM banks):
    dcopy (d overwrites the consumed x block), square, y-mul on vector;
    rsqrt (Abs_reciprocal_sqrt) on scalar.
  - Software pipelining: the loop issues prologue(s+1), d-pass(s),
    v-pass(s-1), so every engine's FIFO queue only sees ready work.
"""

import sys

for _p in ("/opt/trn_rl_repo", "/root/.axon_site/_ro/trn_rl_repo"):
    if _p not in sys.path:
        sys.path.append(_p)

import numpy as np

import concourse.bacc as bacc
import concourse.mybir as mybir
from concourse.tile import TileContext
from concourse import bass_utils

N_ROWS = 8192
L_FULL = 4096
N_CORES = 8
L_SHARD = L_FULL // N_CORES

AFWD = 0.999
EPS = 1e-05
B = 127           # time steps per block (partition 127 = carry row)
SB = 16           # blocks per super-block
NSB = 4           # super-blocks covering blocks 0..63
TAIL_ROWS = N_ROWS - 64 * B  # 64
CW = SB * 512     # 8192 cols per slab

F32 = mybir.dt.float32
F16 = mybir.dt.float16
AF = mybir.ActivationFunctionType


def _build_weights():
    A = AFWD
    Abig = A ** B
    # WD [128,128]: [j,k] = coeff of moving row j for output col k.
    WD = np.zeros((128, 128))
    for k in range(B):
        WD[k, k] = 1.0
        for j in range(k):
            WD[j, k] = -(1 - A) * A ** (k - 1 - j)
        WD[127, k] = -(A ** k)
    for j in range(B):
        WD[j, 127] = (1 - A) * A ** (126 - j)
    WD[127, 127] = Abig

    # TV [128,128]: v^(k) = a^k*nu + (1-a) sum_{j<k} a^(k-j) e_j
    TV = np.zeros((128, 128))
    for k in range(B):
        for j in range(k):
            TV[j, k] = (1 - A) * A ** (k - j)
        TV[127, k] = A ** k
    for j in range(B):
        TV[j, 127] = (1 - A) * A ** (127 - j)
    TV[127, 127] = Abig

    # Per-block mu / var increment row vectors.
    wm = np.array([(1 - A) * A ** (126 - j) for j in range(B)])
    wmv = np.array([(1 - A) * A ** (127 - j) for j in range(B)])

    # Fused scan stationaries WSM/WSV [127, 16*17]: slot i (block i within
    # the super-block) is a [127,17] stationary.  Output rows of the
    # accumulated [17,512] PSUM tile:
    #   row 0      = next carry  = Abig^16 c + sum_i Abig^(15-i) m(i)
    #   row r=1..16: carry of block 16s+(r-1)
    #              = Abig^(r-1) c + sum_{i<=r-2} Abig^(r-2-i) m(i)
    def scan_cols(i, w):
        # [127, 17] scan stationary for block i of the super-block
        cols = np.zeros((B, SB + 1))
        cols[:, 0] = Abig ** (SB - 1 - i) * w
        for r in range(i + 2, SB + 1):
            cols[:, r] = Abig ** (r - 2 - i) * w
        return cols

    # Col-tiled rounds: round r runs blocks {4r+g} concurrently in PE
    # col-groups g=0..3 -> stationary [127, 128], group g at cols 32g..32g+16
    WSM4 = np.zeros((B, 4 * 128))
    WSV4 = np.zeros((B, 4 * 128))
    for r in range(4):
        for g in range(4):
            i = 4 * r + g
            WSM4[:, 128 * r + 32 * g:128 * r + 32 * g + SB + 1] = \
                scan_cols(i, wm)
            WSV4[:, 128 * r + 32 * g:128 * r + 32 * g + SB + 1] = \
                scan_cols(i, wmv)

    # Combine: total carries = sum over the 4 groups' partial scans.
    SEL = np.zeros((128, SB + 1))
    for g in range(4):
        for c in range(SB + 1):
            SEL[32 * g + c, c] = 1.0

    # Carry inject stationary IC [1, 17].
    IC = np.zeros((1, SB + 1))
    IC[0, 0] = Abig ** SB
    for r in range(1, SB + 1):
        IC[0, r] = Abig ** (r - 1)
    return {"wd": WD, "tv": TV, "wsm4": WSM4, "wsv4": WSV4, "ic": IC,
            "sel": SEL}


_WEIGHTS = {k: np.ascontiguousarray(v.astype(np.float16))
            for k, v in _build_weights().items()}

# Pack all weights into one fp16 tensor (single init DMA).
_WCOLS = {"wd": (0, 128), "tv": (128, 128), "wsm4": (256, 512),
          "wsv4": (768, 512), "ic": (1280, 17), "sel": (1297, 17)}
_WPACK = np.zeros((128, 1314), dtype=np.float16)
for _n, (_c0, _w) in _WCOLS.items():
    _arr = _WEIGHTS[_n]
    _WPACK[:_arr.shape[0], _c0:_c0 + _arr.shape[1]] = _arr
_WPACK = np.ascontiguousarray(_WPACK)


def _build_nc(l_cols: int):
    nc = bacc.Bacc()
    x = nc.declare_dram_parameter("x", [5 * 128, CW], F16, isOutput=False)
    mu0 = nc.declare_dram_parameter("mu0", [1, l_cols], F16, isOutput=False)
    var0 = nc.declare_dram_parameter("var0", [1, l_cols], F16, isOutput=False)
    wpack = nc.declare_dram_parameter("wpack", [128, 1314], F16,
                                      isOutput=False)
    y = nc.declare_dram_parameter("y", [5 * 128, CW], F16, isOutput=True)

    with TileContext(nc) as tc:
        with (
            tc.tile_pool(name="consts", bufs=1) as cpool,
            tc.tile_pool(name="xsb", bufs=3) as xsb_pool,
            tc.tile_pool(name="esb", bufs=3) as esb_pool,
            tc.tile_pool(name="ysb", bufs=2) as ysb_pool,
            tc.tile_pool(name="rs", bufs=3) as rs_pool,
            tc.tile_pool(name="ct", bufs=2) as ct_pool,
            tc.tile_pool(name="pcs", bufs=2) as pcs_pool,
            tc.tile_pool(name="cvt", bufs=2) as cvt_pool,
            tc.tile_pool(name="pd", bufs=2, space="PSUM") as pd_pool,
            tc.tile_pool(name="pv", bufs=1, space="PSUM") as pv_pool,
            tc.tile_pool(name="pc", bufs=1, space="PSUM") as pc_pool,
            tc.tile_pool(name="pcv", bufs=1, space="PSUM") as pcv_pool,
        ):
            wall = cpool.tile([128, 1314], F16, tag="wall", name="wall")
            nc.sync.dma_start(out=wall[:, :], in_=wpack[:, :])
            wsb = {}
            for name, (c0, w) in _WCOLS.items():
                rows = _WEIGHTS[name].shape[0]
                wsb[name] = wall[0:rows, c0:c0 + w]
            eps_sb = cpool.tile([128, 1], F32, tag="eps", name="eps_sb")
            nc.vector.memset(eps_sb[:, :], EPS)

            # initial carries (partition 0 of [1,512] tiles)
            cm0 = cpool.tile([1, 512], F16, tag="cm0", name="cm0")
            cv0 = cpool.tile([1, 512], F16, tag="cv0", name="cv0")
            nc.sync.dma_start(out=cm0[0:1, :], in_=mu0[:, :])
            nc.sync.dma_start(out=cv0[0:1, :], in_=var0[:, :])

            # tail block tiles (block 64: rows 8128..8191; host zero-pads)
            xtail = cpool.tile([128, 512], F16, tag="xtail", name="xtail")
            etail = cpool.tile([128, 512], F16, tag="etail", name="etail")
            nc.gpsimd.dma_start(out=xtail[0:128, :],
                                in_=x[4 * 128:5 * 128, 0:512])

            xsbs = [None] * NSB
            esbs = [None] * NSB
            ysbs = [None] * NSB
            cts = [None] * NSB
            cvts = [None] * NSB

            def xload(s):
                xsb = xsb_pool.tile([128, CW], F16, tag="xsb",
                                    name=f"xsb{s}")
                xsbs[s] = xsb
                half = CW // 2
                nc.gpsimd.dma_start(out=xsb[0:128, 0:half],
                                    in_=x[128 * s:128 * s + 128, 0:half])
                nc.gpsimd.dma_start(out=xsb[0:128, half:CW],
                                    in_=x[128 * s:128 * s + 128, half:CW])

            def m_round(s, r, pc):
                xsb = xsbs[s]
                for g in range(4):
                    i = 4 * r + g
                    c0 = 128 * r + 32 * g
                    nc.tensor.matmul(pc[32 * g:32 * g + SB + 1, :],
                                     wsb["wsm4"][:, c0:c0 + SB + 1],
                                     xsb[0:B, 512 * i:512 * i + 512],
                                     start=(r == 0), stop=(r == 3),
                                     tile_position=(0, 32 * g))

            def mu_combine(s, pc):
                pcs = pcs_pool.tile([128, 512], F16, tag="pcs")
                nc.vector.tensor_copy(pcs[0:128, :], pc[0:128, :])
                nc.tensor.matmul(pc[0:SB + 1, :], wsb["sel"][:, :],
                                 pcs[0:128, :], start=True, stop=False)
                carry = cm0 if s == 0 else cts[s - 1]
                nc.tensor.matmul(pc[0:SB + 1, :], wsb["ic"][:, :],
                                 carry[0:1, :], start=False, stop=True)
                ct = ct_pool.tile([SB + 1, 512], F16, tag="ct",
                                  name=f"ct{s}")
                cts[s] = ct
                nc.vector.tensor_copy(ct[0:SB + 1, :], pc[0:SB + 1, :])
                nc.sync.dma_start(out=xsbs[s][127:128, 0:CW],
                                  in_=ct[1:SB + 1, :])
                if s == NSB - 1:
                    nc.sync.dma_start(out=xtail[127:128, :], in_=ct[0:1, :])

            def d_pair(s, q):
                xsb, esb = xsbs[s], esbs[s]
                c0 = 1024 * q
                pd = pd_pool.tile([128, 1024], F32, tag="pd")
                nc.tensor.matmul(pd[:, 0:512], wsb["wd"][:, :],
                                 xsb[0:128, c0:c0 + 512],
                                 start=True, stop=True)
                nc.tensor.matmul(pd[:, 512:1024], wsb["wd"][:, :],
                                 xsb[0:128, c0 + 512:c0 + 1024],
                                 start=True, stop=True)
                nc.vector.tensor_copy(xsb[0:128, c0:c0 + 1024],
                                      pd[0:128, :])
                if q % 4 == 0:
                    nc.scalar.activation(esb[0:128, c0:c0 + 1024],
                                         xsb[0:128, c0:c0 + 1024],
                                         AF.Square)
                else:
                    nc.vector.tensor_mul(esb[0:128, c0:c0 + 1024],
                                         xsb[0:128, c0:c0 + 1024],
                                         xsb[0:128, c0:c0 + 1024])

            def vm_round(s, r, pcv):
                esb = esbs[s]
                for g in range(4):
                    i = 4 * r + g
                    c0 = 128 * r + 32 * g
                    nc.tensor.matmul(pcv[32 * g:32 * g + SB + 1, :],
                                     wsb["wsv4"][:, c0:c0 + SB + 1],
                                     esb[0:B, 512 * i:512 * i + 512],
                                     start=(r == 0), stop=(r == 3),
                                     tile_position=(0, 32 * g))

            def vm_combine(s, pcv):
                pcvs = pcs_pool.tile([128, 512], F16, tag="pcs")
                nc.vector.tensor_copy(pcvs[0:128, :], pcv[0:128, :])
                nc.tensor.matmul(pcv[0:SB + 1, :], wsb["sel"][:, :],
                                 pcvs[0:128, :], start=True, stop=False)
                carry = cv0 if s == 0 else cvts[s - 1]
                nc.tensor.matmul(pcv[0:SB + 1, :], wsb["ic"][:, :],
                                 carry[0:1, :], start=False, stop=True)
                cvt = cvt_pool.tile([SB + 1, 512], F16, tag="cvt",
                                    name=f"cvt{s}")
                cvts[s] = cvt
                nc.vector.tensor_copy(cvt[0:SB + 1, :], pcv[0:SB + 1, :])
                nc.sync.dma_start(out=esbs[s][127:128, 0:CW],
                                  in_=cvt[1:SB + 1, :])
                if s == NSB - 1:
                    nc.sync.dma_start(out=etail[127:128, :], in_=cvt[0:1, :])

            def v_pair(s, q):
                xsb, esb, ysb = xsbs[s], esbs[s], ysbs[s]
                c0 = 1024 * q
                pv = pv_pool.tile([128, 1024], F32, tag="pv")
                nc.tensor.matmul(pv[:, 0:512], wsb["tv"][:, :],
                                 esb[0:128, c0:c0 + 512],
                                 start=True, stop=True)
                nc.tensor.matmul(pv[:, 512:1024], wsb["tv"][:, :],
                                 esb[0:128, c0 + 512:c0 + 1024],
                                 start=True, stop=True)
                rs = rs_pool.tile([128, 1024], F16, tag="rs")
                nc.scalar.activation(rs[0:128, :], pv[0:128, :],
                                     AF.Abs_reciprocal_sqrt,
                                     bias=eps_sb[0:128, :])
                nc.vector.tensor_mul(ysb[0:128, c0:c0 + 1024],
                                     xsb[0:128, c0:c0 + 1024],
                                     rs[0:128, :])

            # ---------- interleaved pipeline ----------
            xload(0)
            pc0 = pc_pool.tile([128, 512], F32, tag="pc", name="pc0")
            for r in range(4):
                m_round(0, r, pc0)
            mu_combine(0, pc0)
            for s in range(NSB + 1):
                dp = s < NSB
                vp = s >= 1
                if dp:
                    if s + 1 < NSB:
                        xload(s + 1)
                    esb = esb_pool.tile([128, CW], F16, tag="esb",
                                        name=f"esb{s}")
                    esbs[s] = esb
                    pcv = pcv_pool.tile([128, 512], F32, tag="pcv",
                                        name=f"pcv{s}")
                    if s + 1 < NSB:
                        pc = pc_pool.tile([128, 512], F32, tag="pc",
                                          name=f"pc{s + 1}")
                if vp:
                    ysb = ysb_pool.tile([128, CW], F16, tag="ysb",
                                        name=f"ysb{s - 1}")
                    ysbs[s - 1] = ysb
                for q in range(SB // 2):
                    if dp:
                        d_pair(s, q)
                        if s + 1 < NSB and q % 2 == 1:
                            m_round(s + 1, q // 2, pc)
                        if q in (2, 4, 6):
                            vm_round(s, q // 2 - 1, pcv)
                    if vp:
                        v_pair(s - 1, q)
                        if q == 3:
                            nc.sync.dma_start(
                                out=y[128 * (s - 1):128 * (s - 1) + 128,
                                      0:CW // 2],
                                in_=ysb[0:128, 0:CW // 2])
                if dp:
                    vm_round(s, 3, pcv)
                    if s + 1 < NSB:
                        mu_combine(s + 1, pc)
                    vm_combine(s, pcv)
                    if s == NSB - 1:
                        # tail d-chain can start once its mu carry landed
                        pdt = pd_pool.tile([128, 1024], F32, tag="pd",
                                           name="pdt")
                        nc.tensor.matmul(pdt[:, 0:512], wsb["wd"][:, :],
                                         xtail[0:128, :],
                                         start=True, stop=True)
                        nc.vector.tensor_copy(xtail[0:128, :],
                                              pdt[0:128, 0:512])
                        # etail row 127 holds the var carry -- don't clobber
                        nc.vector.tensor_mul(etail[0:B, :], xtail[0:B, :],
                                             xtail[0:B, :])
                if vp:
                    nc.sync.dma_start(
                        out=y[128 * (s - 1):128 * (s - 1) + 128, CW // 2:CW],
                        in_=ysb[0:128, CW // 2:CW])

            # ---------- tail v-chain ----------
            pvt = pv_pool.tile([128, 1024], F32, tag="pv", name="pvt")
            nc.tensor.matmul(pvt[:, 0:512], wsb["tv"][:, :], etail[0:128, :],
                             start=True, stop=True)
            rst = rs_pool.tile([128, 1024], F16, tag="rs", name="rst")
            nc.scalar.activation(rst[0:128, 0:512], pvt[0:128, 0:512],
                                 AF.Abs_reciprocal_sqrt,
                                 bias=eps_sb[0:128, :])
            ytail = cpool.tile([128, 512], F16, tag="ytail", name="ytail")
            nc.vector.tensor_mul(ytail[0:128, :], xtail[0:128, :],
                                 rst[0:128, 0:512])
            nc.sync.dma_start(out=y[4 * 128:5 * 128, 0:512],
                              in_=ytail[0:128, :])

    nc.compile()
    return nc


_NC_CACHE = {}


def _get_nc():
    key = L_SHARD
    if key not in _NC_CACHE:
        _NC_CACHE[key] = _build_nc(key)
    return _NC_CACHE[key]


def kernel(x, mu0, var0, _want_time=False, _trace=False):
    x = np.asarray(x)
    mu0 = np.asarray(mu0).reshape(1, -1)
    var0 = np.asarray(var0).reshape(1, -1)
    assert x.shape == (N_ROWS, L_FULL), x.shape

    nc = _get_nc()
    xf16 = x.astype(np.float16)  # [8192, 4096]
    in_maps = []
    for c in range(N_CORES):
        sl = slice(c * L_SHARD, (c + 1) * L_SHARD)
        xc = xf16[:, sl]  # [8192, 512]
        xdev = np.zeros((5 * 128, CW), dtype=np.float16)
        # slab s row k col 512i+f = xc[127*(16s+i)+k, f]
        m = xc[:64 * B].reshape(NSB, SB, B, L_SHARD).transpose(0, 2, 1, 3)
        xdev.reshape(5, 128, CW)[:NSB, :B, :] = m.reshape(NSB, B, CW)
        xdev[4 * 128:4 * 128 + TAIL_ROWS, 0:512] = xc[64 * B:]
        in_maps.append({
            "x": xdev,
            "mu0": np.ascontiguousarray(mu0[:, sl]).astype(np.float16),
            "var0": np.ascontiguousarray(var0[:, sl]).astype(np.float16),
            "wpack": _WPACK,
        })

    exec_ns = None
    if _trace:
        orig_upload = bass_utils.upload_artifacts
        bass_utils.upload_artifacts = lambda tmpdir: "(skipped)"
        try:
            res = bass_utils.run_bass_kernel_spmd(
                nc, in_maps, list(range(N_CORES)), trace=True
            )
            exec_ns = res.exec_time_ns
        finally:
            bass_utils.upload_artifacts = orig_upload
    else:
        res = bass_utils.run_bass_kernel_spmd(nc, in_maps, list(range(N_CORES)))

    out = np.empty((N_ROWS, L_FULL), dtype=np.float32)
    for c in range(N_CORES):
        sl = slice(c * L_SHARD, (c + 1) * L_SHARD)
        ydev = res.results[c]["y"]  # [640, 8192] f16
        m = ydev.reshape(5, 128, SB, L_SHARD)[:NSB, :B]  # [s, k, i, f]
        out[:64 * B, sl] = m.transpose(0, 2, 1, 3).reshape(64 * B, L_SHARD)
        out[64 * B:, sl] = ydev[4 * 128:4 * 128 + TAIL_ROWS, 0:512]
    if _want_time:
        return out, exec_ns
    return out


# revision 26
# speedup vs baseline: 4.2234x; 1.1062x over previous
"""Online Normalization forward (nn_Norm1d) on 8 Trainium2 NeuronCores.

Reference recurrence over the batch dim t (per feature, sequential):
    d_t   = x_t - mu_t
    y_t   = d_t / sqrt(var_t + eps)
    mu_{t+1}  = mu_t + (1-a)*d_t
    var_{t+1} = a*var_t + a*(1-a)*d_t^2

Sharding: tensor-parallel over the feature dim L (4096 -> 8 x 512); each
feature's scan over N=8192 is independent -> no cross-core communication.

Kernel structure (per core, 512 features):
  - fp16 I/O, host-side cast + block-slab relayout: x and y live in DRAM as
    5 slabs of [128, 8192]: slab s row k col 512*i+f = x[127*(16s+i)+k, f],
    so every bulk DMA is a full-128-partition contiguous transfer (the
    16-engine descriptor spray path).
  - 127-step blocks: time on partitions 0..126, the running carry (mu or
    var) rides partition 127 of the same moving tile; one [128,128]
    stationary computes all 127 d's (or var's) of a block in one matmul.
  - Block-to-block carries come from a fused block-level scan: per block,
    one extra matmul with a scan-weighted stationary (cols = all 16 carry
    outputs of the super-block + next carry at col 0) accumulates into a
    [17,512] PSUM tile; a K=1 inject matmul adds the incoming carry; one
    DVE copy + one SBUF->SBUF scatter DMA plant the carries into partition
    127 of the x/e tiles.  Same machinery for mu (from x) and var (from
    e=d^2; the var pass trails one super-block so nothing ever waits).
  - Elementwise ops run pair-wide (FD=1024 over two adjacent PSUM banks):
    dcopy (d overwrites the consumed x block), square, y-mul on vector;
    rsqrt (Abs_reciprocal_sqrt) on scalar.
  - Software pipelining: the loop issues prologue(s+1), d-pass(s),
    v-pass(s-1), so every engine's FIFO queue only sees ready work.
"""

import sys

for _p in ("/opt/trn_rl_repo", "/root/.axon_site/_ro/trn_rl_repo"):
    if _p not in sys.path:
        sys.path.append(_p)

import numpy as np

import concourse.bacc as bacc
import concourse.mybir as mybir
from concourse.tile import TileContext
from concourse import bass_utils

N_ROWS = 8192
L_FULL = 4096
N_CORES = 8
L_SHARD = L_FULL // N_CORES

AFWD = 0.999
EPS = 1e-05
B = 127           # time steps per block (partition 127 = carry row)
SB = 16           # blocks per super-block
NSB = 4           # super-blocks covering blocks 0..63
TAIL_ROWS = N_ROWS - 64 * B  # 64
CW = SB * 512     # 8192 cols per slab

F32 = mybir.dt.float32
F16 = mybir.dt.float16
AF = mybir.ActivationFunctionType


def _build_weights():
    A = AFWD
    Abig = A ** B
    # WD [128,128]: [j,k] = coeff of moving row j for output col k.
    WD = np.zeros((128, 128))
    for k in range(B):
        WD[k, k] = 1.0
        for j in range(k):
            WD[j, k] = -(1 - A) * A ** (k - 1 - j)
        WD[127, k] = -(A ** k)
    for j in range(B):
        WD[j, 127] = (1 - A) * A ** (126 - j)
    WD[127, 127] = Abig

    # TV [128,128]: v^(k) = a^k*nu + (1-a) sum_{j<k} a^(k-j) e_j
    TV = np.zeros((128, 128))
    for k in range(B):
        for j in range(k):
            TV[j, k] = (1 - A) * A ** (k - j)
        TV[127, k] = A ** k
    for j in range(B):
        TV[j, 127] = (1 - A) * A ** (127 - j)
    TV[127, 127] = Abig

    # Per-block mu / var increment row vectors.
    wm = np.array([(1 - A) * A ** (126 - j) for j in range(B)])
    wmv = np.array([(1 - A) * A ** (127 - j) for j in range(B)])

    # Fused scan stationaries WSM/WSV [127, 16*17]: slot i (block i within
    # the super-block) is a [127,17] stationary.  Output rows of the
    # accumulated [17,512] PSUM tile:
    #   row 0      = next carry  = Abig^16 c + sum_i Abig^(15-i) m(i)
    #   row r=1..16: carry of block 16s+(r-1)
    #              = Abig^(r-1) c + sum_{i<=r-2} Abig^(r-2-i) m(i)
    def scan_cols(i, w):
        # [127, 17] scan stationary for block i of the super-block
        cols = np.zeros((B, SB + 1))
        cols[:, 0] = Abig ** (SB - 1 - i) * w
        for r in range(i + 2, SB + 1):
            cols[:, r] = Abig ** (r - 2 - i) * w
        return cols

    # Col-tiled rounds: round r runs blocks {4r+g} concurrently in PE
    # col-groups g=0..3 -> stationary [127, 128], group g at cols 32g..32g+16
    WSM4 = np.zeros((B, 4 * 128))
    WSV4 = np.zeros((B, 4 * 128))
    for r in range(4):
        for g in range(4):
            i = 4 * r + g
            WSM4[:, 128 * r + 32 * g:128 * r + 32 * g + SB + 1] = \
                scan_cols(i, wm)
            WSV4[:, 128 * r + 32 * g:128 * r + 32 * g + SB + 1] = \
                scan_cols(i, wmv)

    # Combine: total carries = sum over the 4 groups' partial scans.
    SEL = np.zeros((128, SB + 1))
    for g in range(4):
        for c in range(SB + 1):
            SEL[32 * g + c, c] = 1.0

    # Carry inject stationary IC [1, 17].
    IC = np.zeros((1, SB + 1))
    IC[0, 0] = Abig ** SB
    for r in range(1, SB + 1):
        IC[0, r] = Abig ** (r - 1)
    return {"wd": WD, "tv": TV, "wsm4": WSM4, "wsv4": WSV4, "ic": IC,
            "sel": SEL}


_WEIGHTS = {k: np.ascontiguousarray(v.astype(np.float16))
            for k, v in _build_weights().items()}

# Pack all weights into one fp16 tensor (single init DMA).
_WCOLS = {"wd": (0, 128), "tv": (128, 128), "wsm4": (256, 512),
          "wsv4": (768, 512), "ic": (1280, 17), "sel": (1297, 17)}
_WPACK = np.zeros((128, 1314), dtype=np.float16)
for _n, (_c0, _w) in _WCOLS.items():
    _arr = _WEIGHTS[_n]
    _WPACK[:_arr.shape[0], _c0:_c0 + _arr.shape[1]] = _arr
_WPACK = np.ascontiguousarray(_WPACK)


def _build_nc(l_cols: int):
    nc = bacc.Bacc()
    x = nc.declare_dram_parameter("x", [5 * 128, CW], F16, isOutput=False)
    mu0 = nc.declare_dram_parameter("mu0", [1, l_cols], F16, isOutput=False)
    var0 = nc.declare_dram_parameter("var0", [1, l_cols], F16, isOutput=False)
    wpack = nc.declare_dram_parameter("wpack", [128, 1314], F16,
                                      isOutput=False)
    y = nc.declare_dram_parameter("y", [5 * 128, CW], F16, isOutput=True)

    with TileContext(nc) as tc:
        with (
            tc.tile_pool(name="consts", bufs=1) as cpool,
            tc.tile_pool(name="xsb", bufs=3) as xsb_pool,
            tc.tile_pool(name="esb", bufs=3) as esb_pool,
            tc.tile_pool(name="ysb", bufs=2) as ysb_pool,
            tc.tile_pool(name="rs", bufs=3) as rs_pool,
            tc.tile_pool(name="ct", bufs=2) as ct_pool,
            tc.tile_pool(name="pcs", bufs=2) as pcs_pool,
            tc.tile_pool(name="cvt", bufs=2) as cvt_pool,
            tc.tile_pool(name="pd", bufs=2, space="PSUM") as pd_pool,
            tc.tile_pool(name="pv", bufs=1, space="PSUM") as pv_pool,
            tc.tile_pool(name="pc", bufs=1, space="PSUM") as pc_pool,
            tc.tile_pool(name="pcv", bufs=1, space="PSUM") as pcv_pool,
        ):
            wall = cpool.tile([128, 1314], F16, tag="wall", name="wall")
            nc.sync.dma_start(out=wall[:, :], in_=wpack[:, :])
            wsb = {}
            for name, (c0, w) in _WCOLS.items():
                rows = _WEIGHTS[name].shape[0]
                wsb[name] = wall[0:rows, c0:c0 + w]
            eps_sb = cpool.tile([128, 1], F32, tag="eps", name="eps_sb")
            nc.vector.memset(eps_sb[:, :], EPS)

            # initial carries (partition 0 of [1,512] tiles)
            cm0 = cpool.tile([1, 512], F16, tag="cm0", name="cm0")
            cv0 = cpool.tile([1, 512], F16, tag="cv0", name="cv0")
            nc.sync.dma_start(out=cm0[0:1, :], in_=mu0[:, :])
            nc.sync.dma_start(out=cv0[0:1, :], in_=var0[:, :])

            # tail block tiles (block 64: rows 8128..8191; host zero-pads)
            xtail = cpool.tile([128, 512], F16, tag="xtail", name="xtail")
            etail = cpool.tile([128, 512], F16, tag="etail", name="etail")
            nc.gpsimd.dma_start(out=xtail[0:128, :],
                                in_=x[4 * 128:5 * 128, 0:512])

            xsbs = [None] * NSB
            esbs = [None] * NSB
            ysbs = [None] * NSB
            cts = [None] * NSB
            cvts = [None] * NSB

            def xload(s):
                xsb = xsb_pool.tile([128, CW], F16, tag="xsb",
                                    name=f"xsb{s}")
                xsbs[s] = xsb
                for c in range(4):
                    c0 = (CW // 4) * c
                    c1 = c0 + CW // 4
                    nc.gpsimd.dma_start(out=xsb[0:128, c0:c1],
                                        in_=x[128 * s:128 * s + 128, c0:c1])

            def m_round(s, r, pc):
                xsb = xsbs[s]
                for g in range(4):
                    i = 4 * r + g
                    c0 = 128 * r + 32 * g
                    nc.tensor.matmul(pc[32 * g:32 * g + SB + 1, :],
                                     wsb["wsm4"][:, c0:c0 + SB + 1],
                                     xsb[0:B, 512 * i:512 * i + 512],
                                     start=(r == 0), stop=(r == 3),
                                     tile_position=(0, 32 * g))

            def mu_combine(s, pc):
                pcs = pcs_pool.tile([128, 512], F16, tag="pcs")
                nc.vector.tensor_copy(pcs[0:128, :], pc[0:128, :])
                nc.tensor.matmul(pc[0:SB + 1, :], wsb["sel"][:, :],
                                 pcs[0:128, :], start=True, stop=False)
                carry = cm0 if s == 0 else cts[s - 1]
                nc.tensor.matmul(pc[0:SB + 1, :], wsb["ic"][:, :],
                                 carry[0:1, :], start=False, stop=True)
                ct = ct_pool.tile([SB + 1, 512], F16, tag="ct",
                                  name=f"ct{s}")
                cts[s] = ct
                nc.vector.tensor_copy(ct[0:SB + 1, :], pc[0:SB + 1, :])
                nc.sync.dma_start(out=xsbs[s][127:128, 0:CW],
                                  in_=ct[1:SB + 1, :])
                if s == NSB - 1:
                    nc.sync.dma_start(out=xtail[127:128, :], in_=ct[0:1, :])

            def d_pair(s, q):
                xsb, esb = xsbs[s], esbs[s]
                c0 = 1024 * q
                pd = pd_pool.tile([128, 1024], F32, tag="pd")
                nc.tensor.matmul(pd[:, 0:512], wsb["wd"][:, :],
                                 xsb[0:128, c0:c0 + 512],
                                 start=True, stop=True)
                nc.tensor.matmul(pd[:, 512:1024], wsb["wd"][:, :],
                                 xsb[0:128, c0 + 512:c0 + 1024],
                                 start=True, stop=True)
                nc.vector.tensor_copy(xsb[0:128, c0:c0 + 1024],
                                      pd[0:128, :])
                if q % 4 == 0:
                    nc.scalar.activation(esb[0:128, c0:c0 + 1024],
                                         xsb[0:128, c0:c0 + 1024],
                                         AF.Square)
                else:
                    nc.vector.tensor_mul(esb[0:128, c0:c0 + 1024],
                                         xsb[0:128, c0:c0 + 1024],
                                         xsb[0:128, c0:c0 + 1024])

            def vm_round(s, r, pcv):
                esb = esbs[s]
                for g in range(4):
                    i = 4 * r + g
                    c0 = 128 * r + 32 * g
                    nc.tensor.matmul(pcv[32 * g:32 * g + SB + 1, :],
                                     wsb["wsv4"][:, c0:c0 + SB + 1],
                                     esb[0:B, 512 * i:512 * i + 512],
                                     start=(r == 0), stop=(r == 3),
                                     tile_position=(0, 32 * g))

            def vm_combine(s, pcv):
                pcvs = pcs_pool.tile([128, 512], F16, tag="pcs")
                nc.vector.tensor_copy(pcvs[0:128, :], pcv[0:128, :])
                nc.tensor.matmul(pcv[0:SB + 1, :], wsb["sel"][:, :],
                                 pcvs[0:128, :], start=True, stop=False)
                carry = cv0 if s == 0 else cvts[s - 1]
                nc.tensor.matmul(pcv[0:SB + 1, :], wsb["ic"][:, :],
                                 carry[0:1, :], start=False, stop=True)
                cvt = cvt_pool.tile([SB + 1, 512], F16, tag="cvt",
                                    name=f"cvt{s}")
                cvts[s] = cvt
                nc.vector.tensor_copy(cvt[0:SB + 1, :], pcv[0:SB + 1, :])
                nc.sync.dma_start(out=esbs[s][127:128, 0:CW],
                                  in_=cvt[1:SB + 1, :])
                if s == NSB - 1:
                    nc.sync.dma_start(out=etail[127:128, :], in_=cvt[0:1, :])

            def v_pair(s, q, alt_pool=False):
                xsb, esb, ysb = xsbs[s], esbs[s], ysbs[s]
                c0 = 1024 * q
                pool = pd_pool if alt_pool else pv_pool
                pv = pool.tile([128, 1024], F32, tag="pd" if alt_pool
                               else "pv")
                nc.tensor.matmul(pv[:, 0:512], wsb["tv"][:, :],
                                 esb[0:128, c0:c0 + 512],
                                 start=True, stop=True)
                nc.tensor.matmul(pv[:, 512:1024], wsb["tv"][:, :],
                                 esb[0:128, c0 + 512:c0 + 1024],
                                 start=True, stop=True)
                rs = rs_pool.tile([128, 1024], F16, tag="rs")
                nc.scalar.activation(rs[0:128, :], pv[0:128, :],
                                     AF.Abs_reciprocal_sqrt,
                                     bias=eps_sb[0:128, :])
                nc.vector.tensor_mul(ysb[0:128, c0:c0 + 1024],
                                     xsb[0:128, c0:c0 + 1024],
                                     rs[0:128, :])

            # ---------- interleaved pipeline ----------
            xload(0)
            pc0 = pc_pool.tile([128, 512], F32, tag="pc", name="pc0")
            for r in range(4):
                m_round(0, r, pc0)
            mu_combine(0, pc0)
            for s in range(NSB + 1):
                dp = s < NSB
                vp = s >= 1
                if dp:
                    if s + 1 < NSB:
                        xload(s + 1)
                    esb = esb_pool.tile([128, CW], F16, tag="esb",
                                        name=f"esb{s}")
                    esbs[s] = esb
                    pcv = pcv_pool.tile([128, 512], F32, tag="pcv",
                                        name=f"pcv{s}")
                    if s + 1 < NSB:
                        pc = pc_pool.tile([128, 512], F32, tag="pc",
                                          name=f"pc{s + 1}")
                if vp:
                    ysb = ysb_pool.tile([128, CW], F16, tag="ysb",
                                        name=f"ysb{s - 1}")
                    ysbs[s - 1] = ysb
                for q in range(SB // 2):
                    if dp:
                        d_pair(s, q)
                        if s + 1 < NSB and 2 <= q <= 5:
                            m_round(s + 1, q - 2, pc)
                        if s + 1 < NSB and q == 6:
                            mu_combine(s + 1, pc)
                        if q in (2, 4, 6):
                            vm_round(s, q // 2 - 1, pcv)
                    if vp:
                        v_pair(s - 1, q, alt_pool=(not dp and q % 2 == 1))
                        if q == 3:
                            nc.sync.dma_start(
                                out=y[128 * (s - 1):128 * (s - 1) + 128,
                                      0:CW // 2],
                                in_=ysb[0:128, 0:CW // 2])
                if dp:
                    vm_round(s, 3, pcv)
                    vm_combine(s, pcv)
                    if s == NSB - 1:
                        # tail d-chain can start once its mu carry landed
                        pdt = pd_pool.tile([128, 1024], F32, tag="pd",
                                           name="pdt")
                        nc.tensor.matmul(pdt[:, 0:512], wsb["wd"][:, :],
                                         xtail[0:128, :],
                                         start=True, stop=True)
                        nc.vector.tensor_copy(xtail[0:128, :],
                                              pdt[0:128, 0:512])
                        # etail row 127 holds the var carry -- don't clobber
                        nc.vector.tensor_mul(etail[0:B, :], xtail[0:B, :],
                                             xtail[0:B, :])
                if vp:
                    nc.sync.dma_start(
                        out=y[128 * (s - 1):128 * (s - 1) + 128, CW // 2:CW],
                        in_=ysb[0:128, CW // 2:CW])

            # ---------- tail v-chain ----------
            pvt = pv_pool.tile([128, 1024], F32, tag="pv", name="pvt")
            nc.tensor.matmul(pvt[:, 0:512], wsb["tv"][:, :], etail[0:128, :],
                             start=True, stop=True)
            rst = rs_pool.tile([128, 1024], F16, tag="rs", name="rst")
            nc.scalar.activation(rst[0:128, 0:512], pvt[0:128, 0:512],
                                 AF.Abs_reciprocal_sqrt,
                                 bias=eps_sb[0:128, :])
            ytail = cpool.tile([128, 512], F16, tag="ytail", name="ytail")
            nc.vector.tensor_mul(ytail[0:128, :], xtail[0:128, :],
                                 rst[0:128, 0:512])
            nc.sync.dma_start(out=y[4 * 128:5 * 128, 0:512],
                              in_=ytail[0:128, :])

    nc.compile()
    return nc


_NC_CACHE = {}


def _get_nc():
    key = L_SHARD
    if key not in _NC_CACHE:
        _NC_CACHE[key] = _build_nc(key)
    return _NC_CACHE[key]


def kernel(x, mu0, var0, _want_time=False, _trace=False):
    x = np.asarray(x)
    mu0 = np.asarray(mu0).reshape(1, -1)
    var0 = np.asarray(var0).reshape(1, -1)
    assert x.shape == (N_ROWS, L_FULL), x.shape

    nc = _get_nc()
    xf16 = x.astype(np.float16)  # [8192, 4096]
    in_maps = []
    for c in range(N_CORES):
        sl = slice(c * L_SHARD, (c + 1) * L_SHARD)
        xc = xf16[:, sl]  # [8192, 512]
        xdev = np.zeros((5 * 128, CW), dtype=np.float16)
        # slab s row k col 512i+f = xc[127*(16s+i)+k, f]
        m = xc[:64 * B].reshape(NSB, SB, B, L_SHARD).transpose(0, 2, 1, 3)
        xdev.reshape(5, 128, CW)[:NSB, :B, :] = m.reshape(NSB, B, CW)
        xdev[4 * 128:4 * 128 + TAIL_ROWS, 0:512] = xc[64 * B:]
        in_maps.append({
            "x": xdev,
            "mu0": np.ascontiguousarray(mu0[:, sl]).astype(np.float16),
            "var0": np.ascontiguousarray(var0[:, sl]).astype(np.float16),
            "wpack": _WPACK,
        })

    exec_ns = None
    if _trace:
        orig_upload = bass_utils.upload_artifacts
        bass_utils.upload_artifacts = lambda tmpdir: "(skipped)"
        try:
            res = bass_utils.run_bass_kernel_spmd(
                nc, in_maps, list(range(N_CORES)), trace=True
            )
            exec_ns = res.exec_time_ns
        finally:
            bass_utils.upload_artifacts = orig_upload
    else:
        res = bass_utils.run_bass_kernel_spmd(nc, in_maps, list(range(N_CORES)))

    out = np.empty((N_ROWS, L_FULL), dtype=np.float32)
    for c in range(N_CORES):
        sl = slice(c * L_SHARD, (c + 1) * L_SHARD)
        ydev = res.results[c]["y"]  # [640, 8192] f16
        m = ydev.reshape(5, 128, SB, L_SHARD)[:NSB, :B]  # [s, k, i, f]
        out[:64 * B, sl] = m.transpose(0, 2, 1, 3).reshape(64 * B, L_SHARD)
        out[64 * B:, sl] = ydev[4 * 128:4 * 128 + TAIL_ROWS, 0:512]
    if _want_time:
        return out, exec_ns
    return out


# revision 27
# speedup vs baseline: 4.4543x; 1.0547x over previous
"""Online Normalization forward (nn_Norm1d) on 8 Trainium2 NeuronCores.

Reference recurrence over the batch dim t (per feature, sequential):
    d_t   = x_t - mu_t
    y_t   = d_t / sqrt(var_t + eps)
    mu_{t+1}  = mu_t + (1-a)*d_t
    var_{t+1} = a*var_t + a*(1-a)*d_t^2

Sharding: tensor-parallel over the feature dim L (4096 -> 8 x 512); each
feature's scan over N=8192 is independent -> no cross-core communication.

Kernel structure (per core, 512 features):
  - fp16 I/O, host-side cast + block-slab relayout: x and y live in DRAM as
    5 slabs of [128, 8192]: slab s row k col 512*i+f = x[127*(16s+i)+k, f],
    so every bulk DMA is a full-128-partition contiguous transfer (the
    16-engine descriptor spray path).
  - 127-step blocks: time on partitions 0..126, the running carry (mu or
    var) rides partition 127 of the same moving tile; one [128,128]
    stationary computes all 127 d's (or var's) of a block in one matmul.
  - Block-to-block carries come from a fused block-level scan: per block,
    one extra matmul with a scan-weighted stationary (cols = all 16 carry
    outputs of the super-block + next carry at col 0) accumulates into a
    [17,512] PSUM tile; a K=1 inject matmul adds the incoming carry; one
    DVE copy + one SBUF->SBUF scatter DMA plant the carries into partition
    127 of the x/e tiles.  Same machinery for mu (from x) and var (from
    e=d^2; the var pass trails one super-block so nothing ever waits).
  - Elementwise ops run pair-wide (FD=1024 over two adjacent PSUM banks):
    dcopy (d overwrites the consumed x block), square, y-mul on vector;
    rsqrt (Abs_reciprocal_sqrt) on scalar.
  - Software pipelining: the loop issues prologue(s+1), d-pass(s),
    v-pass(s-1), so every engine's FIFO queue only sees ready work.
"""

import sys

for _p in ("/opt/trn_rl_repo", "/root/.axon_site/_ro/trn_rl_repo"):
    if _p not in sys.path:
        sys.path.append(_p)

import numpy as np

import concourse.bacc as bacc
import concourse.mybir as mybir
from concourse.tile import TileContext
from concourse import bass_utils

N_ROWS = 8192
L_FULL = 4096
N_CORES = 8
L_SHARD = L_FULL // N_CORES

AFWD = 0.999
EPS = 1e-05
B = 127           # time steps per block (partition 127 = carry row)
SB = 16           # blocks per super-block
NSB = 4           # super-blocks covering blocks 0..63
TAIL_ROWS = N_ROWS - 64 * B  # 64
CW = SB * 512     # 8192 cols per slab

F32 = mybir.dt.float32
F16 = mybir.dt.float16
AF = mybir.ActivationFunctionType


def _build_weights():
    A = AFWD
    Abig = A ** B
    # WD [128,128]: [j,k] = coeff of moving row j for output col k.
    WD = np.zeros((128, 128))
    for k in range(B):
        WD[k, k] = 1.0
        for j in range(k):
            WD[j, k] = -(1 - A) * A ** (k - 1 - j)
        WD[127, k] = -(A ** k)
    for j in range(B):
        WD[j, 127] = (1 - A) * A ** (126 - j)
    WD[127, 127] = Abig

    # TV [128,128]: v^(k) = a^k*nu + (1-a) sum_{j<k} a^(k-j) e_j
    TV = np.zeros((128, 128))
    for k in range(B):
        for j in range(k):
            TV[j, k] = (1 - A) * A ** (k - j)
        TV[127, k] = A ** k
    for j in range(B):
        TV[j, 127] = (1 - A) * A ** (127 - j)
    TV[127, 127] = Abig

    # Per-block mu / var increment row vectors.
    wm = np.array([(1 - A) * A ** (126 - j) for j in range(B)])
    wmv = np.array([(1 - A) * A ** (127 - j) for j in range(B)])

    # Fused scan stationaries WSM/WSV [127, 16*17]: slot i (block i within
    # the super-block) is a [127,17] stationary.  Output rows of the
    # accumulated [17,512] PSUM tile:
    #   row 0      = next carry  = Abig^16 c + sum_i Abig^(15-i) m(i)
    #   row r=1..16: carry of block 16s+(r-1)
    #              = Abig^(r-1) c + sum_{i<=r-2} Abig^(r-2-i) m(i)
    def scan_cols(i, w):
        # [127, 17] scan stationary for block i of the super-block
        cols = np.zeros((B, SB + 1))
        cols[:, 0] = Abig ** (SB - 1 - i) * w
        for r in range(i + 2, SB + 1):
            cols[:, r] = Abig ** (r - 2 - i) * w
        return cols

    # Col-tiled rounds: round r runs blocks {4r+g} concurrently in PE
    # col-groups g=0..3 -> stationary [127, 128], group g at cols 32g..32g+16
    WSM4 = np.zeros((B, 4 * 128))
    WSV4 = np.zeros((B, 4 * 128))
    for r in range(4):
        for g in range(4):
            i = 4 * r + g
            WSM4[:, 128 * r + 32 * g:128 * r + 32 * g + SB + 1] = \
                scan_cols(i, wm)
            WSV4[:, 128 * r + 32 * g:128 * r + 32 * g + SB + 1] = \
                scan_cols(i, wmv)

    # Combine: total carries = sum over the 4 groups' partial scans.
    SEL = np.zeros((128, SB + 1))
    for g in range(4):
        for c in range(SB + 1):
            SEL[32 * g + c, c] = 1.0

    # Carry inject stationary IC [1, 17].
    IC = np.zeros((1, SB + 1))
    IC[0, 0] = Abig ** SB
    for r in range(1, SB + 1):
        IC[0, r] = Abig ** (r - 1)
    return {"wd": WD, "tv": TV, "wsm4": WSM4, "wsv4": WSV4, "ic": IC,
            "sel": SEL}


_WEIGHTS = {k: np.ascontiguousarray(v.astype(np.float16))
            for k, v in _build_weights().items()}

# Pack all weights into one fp16 tensor (single init DMA).
_WCOLS = {"wd": (0, 128), "tv": (128, 128), "wsm4": (256, 512),
          "wsv4": (768, 512), "ic": (1280, 17), "sel": (1297, 17)}
_WPACK = np.zeros((128, 1314), dtype=np.float16)
for _n, (_c0, _w) in _WCOLS.items():
    _arr = _WEIGHTS[_n]
    _WPACK[:_arr.shape[0], _c0:_c0 + _arr.shape[1]] = _arr
_WPACK = np.ascontiguousarray(_WPACK)


def _build_nc(l_cols: int):
    nc = bacc.Bacc()
    x = nc.declare_dram_parameter("x", [5 * 128, CW], F16, isOutput=False)
    mu0 = nc.declare_dram_parameter("mu0", [1, l_cols], F16, isOutput=False)
    var0 = nc.declare_dram_parameter("var0", [1, l_cols], F16, isOutput=False)
    wpack = nc.declare_dram_parameter("wpack", [128, 1314], F16,
                                      isOutput=False)
    y = nc.declare_dram_parameter("y", [5 * 128, CW], F16, isOutput=True)

    with TileContext(nc) as tc:
        with (
            tc.tile_pool(name="consts", bufs=1) as cpool,
            tc.tile_pool(name="xsb", bufs=4) as xsb_pool,
            tc.tile_pool(name="esb", bufs=3) as esb_pool,
            tc.tile_pool(name="ysb", bufs=2) as ysb_pool,
            tc.tile_pool(name="rs", bufs=3) as rs_pool,
            tc.tile_pool(name="ct", bufs=2) as ct_pool,
            tc.tile_pool(name="pcs", bufs=2) as pcs_pool,
            tc.tile_pool(name="cvt", bufs=2) as cvt_pool,
            tc.tile_pool(name="pd", bufs=2, space="PSUM") as pd_pool,
            tc.tile_pool(name="pv", bufs=1, space="PSUM") as pv_pool,
            tc.tile_pool(name="pc", bufs=1, space="PSUM") as pc_pool,
            tc.tile_pool(name="pcv", bufs=1, space="PSUM") as pcv_pool,
        ):
            wall = cpool.tile([128, 1314], F16, tag="wall", name="wall")
            nc.sync.dma_start(out=wall[:, :], in_=wpack[:, :])
            wsb = {}
            for name, (c0, w) in _WCOLS.items():
                rows = _WEIGHTS[name].shape[0]
                wsb[name] = wall[0:rows, c0:c0 + w]
            eps_sb = cpool.tile([128, 1], F32, tag="eps", name="eps_sb")
            nc.vector.memset(eps_sb[:, :], EPS)

            # initial carries (partition 0 of [1,512] tiles)
            cm0 = cpool.tile([1, 512], F16, tag="cm0", name="cm0")
            cv0 = cpool.tile([1, 512], F16, tag="cv0", name="cv0")
            nc.sync.dma_start(out=cm0[0:1, :], in_=mu0[:, :])
            nc.sync.dma_start(out=cv0[0:1, :], in_=var0[:, :])

            # tail block tiles (block 64: rows 8128..8191; host zero-pads)
            xtail = cpool.tile([128, 512], F16, tag="xtail", name="xtail")
            etail = cpool.tile([128, 512], F16, tag="etail", name="etail")
            nc.gpsimd.dma_start(out=xtail[0:128, :],
                                in_=x[4 * 128:5 * 128, 0:512])

            xsbs = [None] * NSB
            esbs = [None] * NSB
            ysbs = [None] * NSB
            cts = [None] * NSB
            cvts = [None] * NSB

            def xload(s):
                xsb = xsb_pool.tile([128, CW], F16, tag="xsb",
                                    name=f"xsb{s}")
                xsbs[s] = xsb
                for c in range(4):
                    c0 = (CW // 4) * c
                    c1 = c0 + CW // 4
                    nc.gpsimd.dma_start(out=xsb[0:128, c0:c1],
                                        in_=x[128 * s:128 * s + 128, c0:c1])

            def m_round(s, r, pc):
                xsb = xsbs[s]
                for g in range(4):
                    i = 4 * r + g
                    c0 = 128 * r + 32 * g
                    nc.tensor.matmul(pc[32 * g:32 * g + SB + 1, :],
                                     wsb["wsm4"][:, c0:c0 + SB + 1],
                                     xsb[0:B, 512 * i:512 * i + 512],
                                     start=(r == 0), stop=(r == 3),
                                     tile_position=(0, 32 * g))

            def mu_combine(s, pc):
                pcs = pcs_pool.tile([128, 512], F16, tag="pcs")
                nc.vector.tensor_copy(pcs[0:128, :], pc[0:128, :])
                nc.tensor.matmul(pc[0:SB + 1, :], wsb["sel"][:, :],
                                 pcs[0:128, :], start=True, stop=False)
                carry = cm0 if s == 0 else cts[s - 1]
                nc.tensor.matmul(pc[0:SB + 1, :], wsb["ic"][:, :],
                                 carry[0:1, :], start=False, stop=True)
                ct = ct_pool.tile([SB + 1, 512], F16, tag="ct",
                                  name=f"ct{s}")
                cts[s] = ct
                nc.vector.tensor_copy(ct[0:SB + 1, :], pc[0:SB + 1, :])
                nc.sync.dma_start(out=xsbs[s][127:128, 0:CW],
                                  in_=ct[1:SB + 1, :])
                if s == NSB - 1:
                    nc.sync.dma_start(out=xtail[127:128, :], in_=ct[0:1, :])

            def d_pair(s, q):
                xsb, esb = xsbs[s], esbs[s]
                c0 = 1024 * q
                pd = pd_pool.tile([128, 1024], F32, tag="pd")
                nc.tensor.matmul(pd[:, 0:512], wsb["wd"][:, :],
                                 xsb[0:128, c0:c0 + 512],
                                 start=True, stop=True)
                nc.tensor.matmul(pd[:, 512:1024], wsb["wd"][:, :],
                                 xsb[0:128, c0 + 512:c0 + 1024],
                                 start=True, stop=True)
                nc.vector.tensor_copy(xsb[0:128, c0:c0 + 1024],
                                      pd[0:128, :])
                if q % 2 == 0:
                    nc.scalar.activation(esb[0:128, c0:c0 + 1024],
                                         xsb[0:128, c0:c0 + 1024],
                                         AF.Square)
                else:
                    nc.vector.tensor_mul(esb[0:128, c0:c0 + 1024],
                                         xsb[0:128, c0:c0 + 1024],
                                         xsb[0:128, c0:c0 + 1024])

            def vm_round(s, r, pcv):
                esb = esbs[s]
                for g in range(4):
                    i = 4 * r + g
                    c0 = 128 * r + 32 * g
                    nc.tensor.matmul(pcv[32 * g:32 * g + SB + 1, :],
                                     wsb["wsv4"][:, c0:c0 + SB + 1],
                                     esb[0:B, 512 * i:512 * i + 512],
                                     start=(r == 0), stop=(r == 3),
                                     tile_position=(0, 32 * g))

            def vm_combine(s, pcv):
                pcvs = pcs_pool.tile([128, 512], F16, tag="pcs")
                nc.vector.tensor_copy(pcvs[0:128, :], pcv[0:128, :])
                nc.tensor.matmul(pcv[0:SB + 1, :], wsb["sel"][:, :],
                                 pcvs[0:128, :], start=True, stop=False)
                carry = cv0 if s == 0 else cvts[s - 1]
                nc.tensor.matmul(pcv[0:SB + 1, :], wsb["ic"][:, :],
                                 carry[0:1, :], start=False, stop=True)
                cvt = cvt_pool.tile([SB + 1, 512], F16, tag="cvt",
                                    name=f"cvt{s}")
                cvts[s] = cvt
                nc.vector.tensor_copy(cvt[0:SB + 1, :], pcv[0:SB + 1, :])
                nc.sync.dma_start(out=esbs[s][127:128, 0:CW],
                                  in_=cvt[1:SB + 1, :])
                if s == NSB - 1:
                    nc.sync.dma_start(out=etail[127:128, :], in_=cvt[0:1, :])

            def v_pair(s, q, alt_pool=False):
                xsb, esb, ysb = xsbs[s], esbs[s], ysbs[s]
                c0 = 1024 * q
                pool = pd_pool if alt_pool else pv_pool
                pv = pool.tile([128, 1024], F32, tag="pd" if alt_pool
                               else "pv")
                nc.tensor.matmul(pv[:, 0:512], wsb["tv"][:, :],
                                 esb[0:128, c0:c0 + 512],
                                 start=True, stop=True)
                nc.tensor.matmul(pv[:, 512:1024], wsb["tv"][:, :],
                                 esb[0:128, c0 + 512:c0 + 1024],
                                 start=True, stop=True)
                rs = rs_pool.tile([128, 1024], F16, tag="rs")
                nc.scalar.activation(rs[0:128, :], pv[0:128, :],
                                     AF.Abs_reciprocal_sqrt,
                                     bias=eps_sb[0:128, :])
                nc.vector.tensor_mul(ysb[0:128, c0:c0 + 1024],
                                     xsb[0:128, c0:c0 + 1024],
                                     rs[0:128, :])

            # ---------- interleaved pipeline ----------
            xload(0)
            pc0 = pc_pool.tile([128, 512], F32, tag="pc", name="pc0")
            for r in range(4):
                m_round(0, r, pc0)
            mu_combine(0, pc0)
            xload(1)
            for s in range(NSB + 1):
                dp = s < NSB
                vp = s >= 1
                if dp:
                    if s + 2 < NSB:
                        xload(s + 2)
                    esb = esb_pool.tile([128, CW], F16, tag="esb",
                                        name=f"esb{s}")
                    esbs[s] = esb
                    pcv = pcv_pool.tile([128, 512], F32, tag="pcv",
                                        name=f"pcv{s}")
                    if s + 1 < NSB:
                        pc = pc_pool.tile([128, 512], F32, tag="pc",
                                          name=f"pc{s + 1}")
                if vp:
                    ysb = ysb_pool.tile([128, CW], F16, tag="ysb",
                                        name=f"ysb{s - 1}")
                    ysbs[s - 1] = ysb
                for q in range(SB // 2):
                    if dp:
                        d_pair(s, q)
                        if s + 1 < NSB and 2 <= q <= 5:
                            m_round(s + 1, q - 2, pc)
                        if s + 1 < NSB and q == 6:
                            mu_combine(s + 1, pc)
                        if q in (2, 4, 6):
                            vm_round(s, q // 2 - 1, pcv)
                    if vp:
                        v_pair(s - 1, q, alt_pool=(not dp and q % 2 == 1))
                        if q == 3:
                            nc.sync.dma_start(
                                out=y[128 * (s - 1):128 * (s - 1) + 128,
                                      0:CW // 2],
                                in_=ysb[0:128, 0:CW // 2])
                if dp:
                    vm_round(s, 3, pcv)
                    vm_combine(s, pcv)
                    if s == NSB - 1:
                        # tail d-chain can start once its mu carry landed
                        pdt = pd_pool.tile([128, 1024], F32, tag="pd",
                                           name="pdt")
                        nc.tensor.matmul(pdt[:, 0:512], wsb["wd"][:, :],
                                         xtail[0:128, :],
                                         start=True, stop=True)
                        nc.vector.tensor_copy(xtail[0:128, :],
                                              pdt[0:128, 0:512])
                        # etail row 127 holds the var carry -- don't clobber
                        nc.vector.tensor_mul(etail[0:B, :], xtail[0:B, :],
                                             xtail[0:B, :])
                if vp:
                    nc.sync.dma_start(
                        out=y[128 * (s - 1):128 * (s - 1) + 128, CW // 2:CW],
                        in_=ysb[0:128, CW // 2:CW])

            # ---------- tail v-chain ----------
            pvt = pv_pool.tile([128, 1024], F32, tag="pv", name="pvt")
            nc.tensor.matmul(pvt[:, 0:512], wsb["tv"][:, :], etail[0:128, :],
                             start=True, stop=True)
            rst = rs_pool.tile([128, 1024], F16, tag="rs", name="rst")
            nc.scalar.activation(rst[0:128, 0:512], pvt[0:128, 0:512],
                                 AF.Abs_reciprocal_sqrt,
                                 bias=eps_sb[0:128, :])
            ytail = cpool.tile([128, 512], F16, tag="ytail", name="ytail")
            nc.vector.tensor_mul(ytail[0:128, :], xtail[0:128, :],
                                 rst[0:128, 0:512])
            nc.sync.dma_start(out=y[4 * 128:5 * 128, 0:512],
                              in_=ytail[0:128, :])

    nc.compile()
    return nc


_NC_CACHE = {}


def _get_nc():
    key = L_SHARD
    if key not in _NC_CACHE:
        _NC_CACHE[key] = _build_nc(key)
    return _NC_CACHE[key]


def kernel(x, mu0, var0, _want_time=False, _trace=False):
    x = np.asarray(x)
    mu0 = np.asarray(mu0).reshape(1, -1)
    var0 = np.asarray(var0).reshape(1, -1)
    assert x.shape == (N_ROWS, L_FULL), x.shape

    nc = _get_nc()
    xf16 = x.astype(np.float16)  # [8192, 4096]
    in_maps = []
    for c in range(N_CORES):
        sl = slice(c * L_SHARD, (c + 1) * L_SHARD)
        xc = xf16[:, sl]  # [8192, 512]
        xdev = np.zeros((5 * 128, CW), dtype=np.float16)
        # slab s row k col 512i+f = xc[127*(16s+i)+k, f]
        m = xc[:64 * B].reshape(NSB, SB, B, L_SHARD).transpose(0, 2, 1, 3)
        xdev.reshape(5, 128, CW)[:NSB, :B, :] = m.reshape(NSB, B, CW)
        xdev[4 * 128:4 * 128 + TAIL_ROWS, 0:512] = xc[64 * B:]
        in_maps.append({
            "x": xdev,
            "mu0": np.ascontiguousarray(mu0[:, sl]).astype(np.float16),
            "var0": np.ascontiguousarray(var0[:, sl]).astype(np.float16),
            "wpack": _WPACK,
        })

    exec_ns = None
    if _trace:
        orig_upload = bass_utils.upload_artifacts
        bass_utils.upload_artifacts = lambda tmpdir: "(skipped)"
        try:
            res = bass_utils.run_bass_kernel_spmd(
                nc, in_maps, list(range(N_CORES)), trace=True
            )
            exec_ns = res.exec_time_ns
        finally:
            bass_utils.upload_artifacts = orig_upload
    else:
        res = bass_utils.run_bass_kernel_spmd(nc, in_maps, list(range(N_CORES)))

    out = np.empty((N_ROWS, L_FULL), dtype=np.float32)
    for c in range(N_CORES):
        sl = slice(c * L_SHARD, (c + 1) * L_SHARD)
        ydev = res.results[c]["y"]  # [640, 8192] f16
        m = ydev.reshape(5, 128, SB, L_SHARD)[:NSB, :B]  # [s, k, i, f]
        out[:64 * B, sl] = m.transpose(0, 2, 1, 3).reshape(64 * B, L_SHARD)
        out[64 * B:, sl] = ydev[4 * 128:4 * 128 + TAIL_ROWS, 0:512]
    if _want_time:
        return out, exec_ns
    return out
